# revision 50
# baseline (speedup 1.0000x reference)
"""Trainium2 Bass kernel for nn_CLRerHead (CLRNet-style lane-detection head).

Sharding: data-parallel over batch. 32 items -> 8 cores x NB=4 items.

v3: "negative tent" matmul gather with s-packing, corner-folded RC,
exact xi+frac fp16 band-broadcast, software-pipelined tent pair loop,
item-paired attention/FiLM/heads, engine-balanced abs/min/copy placement,
host-side sample permutation enabling run-batched RC matmuls.
"""

import math
import numpy as np
from contextlib import ExitStack

import concourse.bacc as bacc
import concourse.mybir as mybir
import concourse.tile as tile
from concourse import bass_utils

dt = mybir.dt
AF = mybir.ActivationFunctionType
ALU = mybir.AluOpType

# ---------------- static problem config ----------------
IMG_W, IMG_H = 800.0, 320.0
NR, NS, NP, FC = 72, 36, 192, 64
N_STRIPS = NR - 1
ALPHA = IMG_H / IMG_W
SAMPLE_IDX = (np.linspace(0.0, 1.0, NS) * N_STRIPS).astype(np.int64)
PRIOR_FEAT_YS = np.flip(SAMPLE_IDX.astype(np.float32) / N_STRIPS).copy()
PRIOR_YS = np.linspace(1.0, 0.0, NR, dtype=np.float32)

N_CORES = 8
NB = 4
# (H, W, Wp, pack): Wp = x-padded width, pack = s-values per psum group
LEVELS = [(10, 25, 32, 4), (20, 50, 64, 2), (40, 100, 100, 1)]
PCH = [(0, 128), (128, 64)]
FP16 = dt.float16
F32 = dt.float32

Q_S = (1.0 - PRIOR_YS[SAMPLE_IDX[::-1]]).astype(np.float32)
QF_R = (1.0 - PRIOR_YS).astype(np.float32)

GPT = 8  # rc groups per psum tile (8 * 64 cols = 512 f32 = one bank)

# engine balance knobs: emit DVE-abs when pair_idx % mod == 0 (0 = never)
# NOTE: abs_max/mod are NOT valid DVE tensor_scalar ops on real HW (walrus
# ISA check) — keep ABS_DVE_MOD at 0 everywhere.
ABS_DVE_MOD = {0: 0, 1: 0, 2: 0}
# tent-min goes to Pool when pair_idx % mod == 0 (0 = always DVE)
MIN_POOL_MOD = {0: 1, 1: 2, 2: 3}
RC_ACT = {0: False, 1: False, 2: False}   # rc copy: True=ACT False=DVE None=alt
HEADS_DVE = False                          # head bias+relu on DVE instead of ACT
# gather precision: "fp16" | "dual" (hi+lo fp16, ~21-bit, 2 matmuls/group)
GATHER = "f32"


def _s_of_t(li):
    """Sample index s for transposed slot t (band-major packing)."""
    H, W, Wp, pack = LEVELS[li]
    G = NS // pack
    t = np.arange(NS)
    return (t % pack) * G + t // pack


def _level_ytab(H):
    ys = PRIOR_FEAT_YS * (H - 1)
    y0 = np.clip(np.floor(ys).astype(np.int64), 0, H - 1)
    y1 = np.minimum(y0 + 1, H - 1)
    wy1 = (ys - y0).astype(np.float32)
    wy1 = np.where(y1 == y0, 0.0, wy1).astype(np.float32)
    wy0 = (1.0 - wy1).astype(np.float32)
    return y0, y1, wy0, wy1


def _nwfc2(W_fc, H):
    """[128, NS*64] fp16: rows corner*64+ch = -wy_corner[s] * Wfc[ch*NS+s, d]."""
    _, _, wy0, wy1 = _level_ytab(H)
    out = np.zeros((128, NS * 64), np.float32)
    for s in range(NS):
        blk = W_fc[s::NS, :]  # (64ch, 64d)
        out[0:64, s * 64:(s + 1) * 64] = -wy0[s] * blk
        out[64:128, s * 64:(s + 1) * 64] = -wy1[s] * blk
    return out.astype(np.float16)


def _bandc(Wp, pack):
    """Band const [96, nvar*128] fp16 replicated at row bases 0/32/64."""
    nvar = 16 // pack
    one = np.zeros((32, nvar * 128), np.float32)
    for v in range(nvar):
        for j in range(pack):
            r = 2 * (v * pack + j)
            one[r, v * 128 + j * Wp:v * 128 + (j + 1) * Wp] = 1.0
            one[r + 1, v * 128 + j * Wp:v * 128 + (j + 1) * Wp] = 1.0
    return np.concatenate([one, one, one], axis=0).astype(np.float16)


_CACHE = {}

F32_CONSTS = [
    ("qrep0", 128, NS), ("qrep1", 128, NS), ("qrep2", 128, NS),
    ("qfrep", 128, NR), ("halfpi", 128, 1),
    ("negiota0", 128, 1), ("negiota1", 128, 1), ("negiota2", 128, 1),
    ("ident", 128, 128),
    ("W_t1", 64, 256), ("b_t1", 128, 2),
    ("W_t2a", 128, 256), ("W_t2b", 128, 256), ("b_t2", 128, 2),
    ("W_sta", 128, 128), ("W_stb", 128, 128), ("bstS1", 64, 1), ("bstSh", 64, 1),
    ("W_tca", 128, 64), ("W_tcb", 128, 64), ("b_tc", 64, 1),
    ("b_fc", 64, 1), ("b_c1", 64, 1), ("b_c2", 64, 1),
    ("b_r1", 64, 1), ("b_r2", 64, 1), ("b_cls", 2, 1), ("b_reg", 76, 1),
    # f32 head weights: the reg head feeds anchor updates whose xs
    # sensitivity is ~25x, so this chain stays in f32
    ("Wf_c1", 64, 64), ("Wf_c2", 64, 64), ("Wf_r1", 64, 64),
    ("Wf_r2", 64, 64), ("Wf_cls", 64, 2), ("Wf_reg", 64, 76),
    ("Wf_q", 64, 64), ("Wf_k", 64, 64), ("Wf_v", 64, 64),
]
# fp16 consts split into two packs: hpak0 needed first (level 2 + attention)
F16A_CONSTS = [
    ("nwfc2_0", 128, NS * 64),
    ("bandc0", 96, (16 // LEVELS[0][3]) * 128),
    ("W_q", 64, 64), ("W_k", 64, 64), ("W_v", 64, 64), ("W_o", 64, 64),
    ("W_c1", 64, 64), ("W_c2", 64, 64), ("W_r1", 64, 64), ("W_r2", 64, 64),
    ("W_cls", 64, 2), ("W_reg", 64, 76), ("ones16", 128, 1),
]
F16B_CONSTS = [
    ("nwfc2_1", 128, NS * 64), ("nwfc2_2", 128, NS * 64),
    ("bandc1", 96, (16 // LEVELS[1][3]) * 128),
    ("bandc2", 96, (16 // LEVELS[2][3]) * 128),
]


def _pack_offsets(spec):
    offs, col = {}, 0
    for name, rows, cols in spec:
        offs[name] = (col, rows, cols)
        col += cols
    return offs, col


F32_OFFS, F32_NCOL = _pack_offsets(F32_CONSTS)
F16A_OFFS, F16A_NCOL = _pack_offsets(F16A_CONSTS)
F16B_OFFS, F16B_NCOL = _pack_offsets(F16B_CONSTS)


def _build_program(num_devices=N_CORES):
    nc = bacc.Bacc("TRN2", target_bir_lowering=False, debug=False,
                   num_devices=num_devices)
    D = {}

    def din(name, shape, dtype=F32):
        D[name] = nc.dram_tensor(name, list(shape), dtype, kind="ExternalInput")

    din("cpak", (128, F32_NCOL))
    din("hpak0", (128, F16A_NCOL), FP16)
    din("hpak1", (128, F16B_NCOL), FP16)
    for li, (H, W, Wp, pack) in enumerate(LEVELS):
        din(f"festack{li}", (128, NB * H * Wp), FP16)
    din("anch0", (128, NB * 3))
    din("anch1", (64, NB * 3))
    din("sinargsT", (64, NB))

    out_t = nc.dram_tensor("out", [NB, NP, 78], F32, kind="ExternalOutput")

    with tile.TileContext(nc) as tc, ExitStack() as ex:
        cpool = ex.enter_context(tc.tile_pool(name="consts", bufs=1))
        state = ex.enter_context(tc.tile_pool(name="state", bufs=1))
        wk = ex.enter_context(tc.tile_pool(name="work", bufs=2))
        big = ex.enter_context(tc.tile_pool(name="big", bufs=2))
        psA = ex.enter_context(tc.tile_pool(name="psA", bufs=3, space="PSUM"))
        psB = ex.enter_context(tc.tile_pool(name="psB", bufs=2, space="PSUM"))
        psC = ex.enter_context(tc.tile_pool(name="psC", bufs=1, space="PSUM"))
        psD = ex.enter_context(tc.tile_pool(name="psD", bufs=2, space="PSUM"))

        cpak = cpool.tile([128, F32_NCOL], F32, tag="cpak", name="cpak")
        hpak0 = cpool.tile([128, F16A_NCOL], FP16, tag="hpak0", name="hpak0")
        hpak1 = cpool.tile([128, F16B_NCOL], FP16, tag="hpak1", name="hpak1")
        nc.sync.dma_start(cpak[:], D["cpak"].ap())

        def Cf(name, p0=0, pn=None, c0=0, cn=None):
            off, rows, cols = F32_OFFS[name]
            pn = rows if pn is None else pn
            cn = cols if cn is None else cn
            return cpak[p0:p0 + pn, off + c0:off + c0 + cn]

        def Ch(name, p0=0, pn=None, c0=0, cn=None):
            if name in F16A_OFFS:
                off, rows, cols = F16A_OFFS[name]
                pak = hpak0
            else:
                off, rows, cols = F16B_OFFS[name]
                pak = hpak1
            pn = rows if pn is None else pn
            cn = cols if cn is None else cn
            return pak[p0:p0 + pn, off + c0:off + c0 + cn]

        anch = []
        for ci, (p0, pn) in enumerate(PCH):
            a = state.tile([pn, NB * 3], F32, tag=f"anch{ci}", name=f"anch{ci}")
            nc.sync.dma_start(a[:], D[f"anch{ci}"].ap())
            anch.append(a)
        sarg = cpool.tile([64, NB], F32, tag="sarg", name="sarg")
        nc.sync.dma_start(sarg[:], D["sinargsT"].ap())
        nc.sync.dma_start(hpak0[:], D["hpak0"].ap())

        fst = []
        for li, (H, W, Wp, pack) in enumerate(LEVELS):
            t = cpool.tile([128, NB * H * Wp], FP16, tag=f"fst{li}",
                           name=f"fst{li}")
            nc.sync.dma_start(t[:], D[f"festack{li}"].ap())
            fst.append(t)
            if li == 0:
                nc.sync.dma_start(hpak1[:], D["hpak1"].ap())

        osts = {}
        for b in range(NB):
            for ci, (p0, pn) in enumerate(PCH):
                osts[(b, ci)] = state.tile([pn, 78], F32, tag=f"ost{b}_{ci}",
                                           name=f"ost{b}_{ci}")

        # ---------------- time MLP ----------------
        sinT = wk.tile([64, NB], F32, tag="tm_sin", name="sinT")
        nc.scalar.activation(sinT[:], sarg[:], AF.Sin)
        emb = []
        for m in range(2):
            p = psD.tile([128, NB], F32, tag="mm", name=f"p_emb{m}")
            nc.tensor.matmul(p[:], Cf("W_t1", 0, 64, m * 128, 128), sinT[:])
            x = state.tile([128, NB], F32, tag=f"emb{m}", name=f"emb{m}")
            nc.scalar.activation(x[:], p[:], AF.Identity,
                                 bias=Cf("b_t1", 0, 128, m, 1))
            sq = wk.tile([128, NB], F32, tag="tm_sq", name=f"sq{m}")
            nc.scalar.activation(sq[:], x[:], AF.Square)
            cu = wk.tile([128, NB], F32, tag="tm_cu", name=f"cu{m}")
            nc.vector.tensor_tensor(cu[:], sq[:], x[:], ALU.mult)
            nc.vector.tensor_scalar(cu[:], cu[:], 0.044715, None, ALU.mult)
            nc.vector.tensor_tensor(cu[:], cu[:], x[:], ALU.add)
            th = wk.tile([128, NB], F32, tag="tm_th", name=f"th{m}")
            nc.scalar.activation(th[:], cu[:], AF.Tanh,
                                 scale=float(np.sqrt(2.0 / np.pi)))
            nc.vector.tensor_scalar(th[:], th[:], 1.0, 0.5, ALU.add, ALU.mult)
            nc.vector.tensor_tensor(x[:], th[:], x[:], ALU.mult)
            emb.append(x)
        tmb = []
        for m in range(2):
            p = psD.tile([128, NB], F32, tag="mm", name=f"p_tmb{m}")
            for k in range(2):
                wt2 = Cf("W_t2a" if k == 0 else "W_t2b", 0, 128, m * 128, 128)
                nc.tensor.matmul(p[:], wt2, emb[k][:],
                                 start=(k == 0), stop=(k == 1))
            x = state.tile([128, NB], F32, tag=f"tmb{m}", name=f"tmb{m}")
            nc.scalar.activation(x[:], p[:], AF.Identity,
                                 bias=Cf("b_t2", 0, 128, m, 1))
            tmb.append(x)
        sil = []
        for m in range(2):
            s = wk.tile([128, NB], F32, tag=f"tm_sil{m}", name=f"sil{m}")
            nc.scalar.activation(s[:], tmb[m][:], AF.Sigmoid)
            nc.vector.tensor_tensor(s[:], s[:], tmb[m][:], ALU.mult)
            sil.append(s)
        scale1T = state.tile([64, NB], F32, tag="scale1T", name="scale1T")
        shiftT = state.tile([64, NB], F32, tag="shiftT", name="shiftT")
        for j, (dst, bias) in enumerate([(scale1T, "bstS1"), (shiftT, "bstSh")]):
            p = psD.tile([64, NB], F32, tag="mm", name=f"p_ss{j}")
            for k in range(2):
                wst = Cf("W_sta" if k == 0 else "W_stb", 0, 128, j * 64, 64)
                nc.tensor.matmul(p[:], wst, sil[k][:],
                                 start=(k == 0), stop=(k == 1))
            nc.scalar.activation(dst[:], p[:], AF.Identity, bias=Cf(bias))
        tokT = state.tile([64, NB], F32, tag="tokT", name="tokT")
        ptk = psD.tile([64, NB], F32, tag="mm", name="p_tok")
        for k in range(2):
            wtc = Cf("W_tca" if k == 0 else "W_tcb")
            nc.tensor.matmul(ptk[:], wtc, tmb[k][:], start=(k == 0), stop=(k == 1))
        nc.scalar.activation(tokT[:], ptk[:], AF.Identity, bias=Cf("b_tc"))

        # ---------------- per-level helpers ----------------
        def gen_ab(li, W, scaleW, c0=0, nb=NB):
            """Batched trig across items [c0, c0+nb): per-chunk (aC,bC,base,g)."""
            res = []
            for ci, (p0, pn) in enumerate(PCH):
                A = anch[ci]
                lo, hi = c0 * 3, (c0 + nb) * 3
                sn = wk.tile([pn, nb], F32, tag=f"sn{ci}", name=f"sn{ci}_{li}_{c0}")
                cs = wk.tile([pn, nb], F32, tag=f"cs{ci}", name=f"cs{ci}_{li}_{c0}")
                nc.scalar.activation(sn[:], A[:, lo + 2:hi:3], AF.Sin,
                                     scale=math.pi)
                nc.scalar.activation(cs[:], A[:, lo + 2:hi:3], AF.Sin,
                                     scale=-math.pi, bias=Cf("halfpi", 0, pn))
                g = wk.tile([pn, nb], F32, tag=f"g{ci}", name=f"g{ci}_{li}_{c0}")
                nc.vector.reciprocal(g[:], sn[:])
                nc.vector.tensor_tensor(g[:], cs[:], g[:], ALU.mult)
                nc.vector.tensor_scalar(g[:], g[:], 1000.0, -1000.0,
                                        ALU.min, ALU.max)
                nc.vector.tensor_scalar(g[:], g[:], ALPHA, None, ALU.mult)
                base = wk.tile([pn, nb], F32, tag=f"bs{ci}", name=f"bs{ci}_{li}_{c0}")
                nc.vector.tensor_tensor(base[:], A[:, lo:hi:3], g[:], ALU.mult)
                nc.vector.tensor_tensor(base[:], A[:, lo + 1:hi:3], base[:],
                                        ALU.subtract)
                aC = wk.tile([pn, nb], F32, tag=f"aC{ci}", name=f"aC{ci}_{li}_{c0}")
                bC = wk.tile([pn, nb], F32, tag=f"bC{ci}", name=f"bC{ci}_{li}_{c0}")
                nc.vector.tensor_scalar(aC[:], base[:], scaleW, None, ALU.mult)
                nc.vector.tensor_scalar(bC[:], g[:], scaleW, None, ALU.mult)
                res.append((aC, bC, base, g))
            return res

        def head_mm(li, wname, bias, src, relu=True, out_p=64, tag="hd"):
            # f32 chain: moving src f32 + f32 stationary weights
            p = psD.tile([128, 2 * NP], F32, tag="mm", name=f"p{tag}_{li}")
            nc.tensor.matmul(p[0:out_p, :], Cf(wname), src[:])
            o = wk.tile([out_p, 2 * NP], F32, tag=f"hd_{tag}",
                        name=f"{tag}o_{li}")
            if HEADS_DVE:
                if relu:
                    nc.vector.tensor_scalar(o[:], p[0:out_p, :],
                                            Cf(bias, 0, out_p), 0.0,
                                            ALU.add, ALU.max)
                else:
                    nc.vector.tensor_scalar(o[:], p[0:out_p, :],
                                            Cf(bias, 0, out_p), None, ALU.add)
            else:
                nc.scalar.activation(o[:], p[0:out_p, :],
                                     AF.Relu if relu else AF.Identity,
                                     bias=Cf(bias, 0, out_p))
            return o

        def emit_heads(li, W, fHp, is_last):
            for bp in range(NB // 2):
                fH = fHp[bp]
                r1 = head_mm(li, "Wf_r1", "b_r1", fH, tag=f"r1{bp}")
                r2 = head_mm(li, "Wf_r2", "b_r2", r1, tag=f"r2{bp}")
                regT = head_mm(li, "Wf_reg", "b_reg", r2, relu=False, out_p=76,
                               tag=f"rg{bp}")
                clsT = None
                if is_last:
                    c1 = head_mm(li, "Wf_c1", "b_c1", fH, tag=f"c1{bp}")
                    c2 = head_mm(li, "Wf_c2", "b_c2", c1, tag=f"c2{bp}")
                    clsT = head_mm(li, "Wf_cls", "b_cls", c2, relu=False,
                                   out_p=2, tag=f"cl{bp}")
                for ci, (p0, pn) in enumerate(PCH):
                    pt2 = psD.tile([128, 2 * 76], F32, tag="mm",
                                   name=f"p_rt{bp}{ci}_{li}")
                    for u in range(2):
                        nc.tensor.transpose(
                            pt2[0:pn, u * 76:u * 76 + 76],
                            regT[:, u * NP + p0:u * NP + p0 + pn],
                            Cf("ident", 0, 76, 0, 76))
                    rn2 = state.tile([pn, 2 * 76], F32, tag=f"regn{bp}_{ci}",
                                     name=f"regn{bp}{ci}_{li}")
                    nc.vector.tensor_copy(rn2[:], pt2[0:pn, :])
                    A = anch[ci]
                    for u in range(2):
                        b = bp * 2 + u
                        nc.vector.tensor_tensor(A[:, b * 3:(b + 1) * 3],
                                                A[:, b * 3:(b + 1) * 3],
                                                rn2[:, u * 76:u * 76 + 3],
                                                ALU.add)
                        if is_last:
                            ost = osts[(b, ci)]
                            nc.vector.tensor_copy(ost[:, 2:5],
                                                  A[:, b * 3:(b + 1) * 3])
                            nc.vector.tensor_copy(ost[:, 5:6],
                                                  rn2[:, u * 76 + 3:u * 76 + 4])
                            _CACHE.setdefault("regn", {})[(b, ci)] = (rn2, u)
                    if is_last:
                        ptc = psD.tile([128, 4], F32, tag="mm",
                                       name=f"p_ct{bp}{ci}")
                        for u in range(2):
                            nc.tensor.transpose(
                                ptc[0:pn, u * 2:u * 2 + 2],
                                clsT[:, u * NP + p0:u * NP + p0 + pn],
                                Cf("ident", 0, 2, 0, 2))
                        for u in range(2):
                            b = bp * 2 + u
                            nc.vector.tensor_copy(osts[(b, ci)][:, 0:2],
                                                  ptc[0:pn, u * 2:u * 2 + 2])
                if is_last:
                    abf = gen_ab(li, W, 1.0, c0=bp * 2, nb=2)
                    for u in range(2):
                        b = bp * 2 + u
                        for ci, (p0, pn) in enumerate(PCH):
                            _, _, base, g = abf[ci]
                            rn2, uu = _CACHE["regn"][(b, ci)]
                            ost = osts[(b, ci)]
                            nc.vector.tensor_scalar(ost[:, 6:78],
                                                    Cf("qfrep", 0, pn),
                                                    g[:, u:u + 1],
                                                    base[:, u:u + 1],
                                                    ALU.mult, ALU.add)
                            nc.vector.tensor_tensor(
                                ost[:, 6:78], ost[:, 6:78],
                                rn2[:, uu * 76 + 4:uu * 76 + 76], ALU.add)
                            nc.sync.dma_start(out_t.ap()[b, p0:p0 + pn, :],
                                              ost[:])

        # ---------------- main loop ----------------
        pending = [None]
        for li, (H, W, Wp, pack) in enumerate(LEVELS):
            G = NS // pack
            nvar = 16 // pack
            is_last = li == len(LEVELS) - 1
            fstL = fst[li]
            ntile = (G + GPT - 1) // GPT
            y0t, _, _, _ = _level_ytab(H)
            PW = pack * Wp
            npairs = (G + 1) // 2

            # --- RC for ALL items first: independent of anchors, fills the
            # level-boundary pipeline bubble ---
            rcsbs_all = {}
            rci = 0
            for b in range(NB):
                for t in range(ntile):
                    g_lo = t * GPT
                    g_hi = min(G, g_lo + GPT)
                    rcp = psB.tile([128, 512], F32, tag="rc",
                                   name=f"rc{b}_{li}_{t}")
                    for j in range(pack):
                        g = g_lo
                        while g < g_hi:
                            s = j * G + g
                            y = int(y0t[s])
                            glen = 1
                            while (g + glen < g_hi
                                   and int(y0t[s + glen]) == y):
                                glen += 1
                            nc.tensor.matmul(
                                rcp[j * Wp:(j + 1) * Wp,
                                    (g - g_lo) * 64:(g - g_lo + glen) * 64],
                                fstL[:, (b * H + y) * Wp:
                                     (b * H + y + 1) * Wp],
                                Ch(f"nwfc2_{li}", 0, 128, s * 64, glen * 64),
                                start=True, stop=True,
                                tile_position=(0, j * Wp))
                            g += glen
                    ng = g_hi - g_lo
                    rcsb = big.tile([128, 512],
                    FP16 if GATHER == "dual" else F32,
                    tag="rcsb", bufs=22,
                                    name=f"rcsb{b}_{li}_{t}")
                    use_act = RC_ACT[li]
                    if use_act is None:
                        use_act = rci % 2 == 0
                    if use_act:
                        nc.scalar.activation(rcsb[0:PW, 0:ng * 64],
                                             rcp[0:PW, 0:ng * 64], AF.Copy)
                    else:
                        nc.vector.tensor_copy(rcsb[0:PW, 0:ng * 64],
                                              rcp[0:PW, 0:ng * 64])
                    rcsl = None
                    if GATHER == "dual":
                        rcsl = big.tile([128, 512], FP16, tag="rcsl", bufs=22,
                                        name=f"rcsl{b}_{li}_{t}")
                        nc.vector.tensor_tensor(rcsl[0:PW, 0:ng * 64],
                                                rcp[0:PW, 0:ng * 64],
                                                rcsb[0:PW, 0:ng * 64],
                                                ALU.subtract)
                    rci += 1
                    rcsbs_all.setdefault(b, []).append((rcsb, rcsl))

            # previous level's heads (anchor updates) overlap this RC block
            if pending[0] is not None:
                pending[0]()
                pending[0] = None

            ab = gen_ab(li, W, float(W - 1))

            # --- phases A+B per item-pair (single shared fps bank:
            # bp0 at partitions 0:64, bp1 at 64:128) ---
            fpsT = psC.tile([128, 2 * NP], F32, tag="fps", name=f"fpsT_{li}")
            fT16s = []
            est2 = {}
            vn2 = {}
            for bp in range(NB // 2):
                r0 = 64 * (bp % 2)
                for u in range(2):
                    b = bp * 2 + u
                    rcsbs = rcsbs_all[b]
                    # A: xf -> xi/frac interleaved -> transpose -> xfif fp16
                    trp = psD.tile([96, NP], F32, tag="mm", name=f"trp{b}_{li}")
                    for ci, (p0, pn) in enumerate(PCH):
                        aC, bC, _, _ = ab[ci]
                        pre = wk.tile([pn, 96], F32, tag=f"pre{ci}", bufs=2,
                                      name=f"pre{b}{ci}_{li}")
                        xf = wk.tile([pn, NS], F32, tag=f"xf{ci}", bufs=2,
                                     name=f"xf{b}{ci}_{li}")
                        nc.vector.tensor_scalar(xf[:], Cf(f"qrep{li}", 0, pn),
                                                bC[:, b:b + 1], aC[:, b:b + 1],
                                                ALU.mult, ALU.add)
                        nc.vector.tensor_scalar(xf[:], xf[:], float(W + 1),
                                                -2.0, ALU.min, ALU.max)
                        # xi = round(xf) via f32 2^23 add/sub; two separate
                        # instructions so the intermediate is rounded to f32
                        # in SBUF (a fused two-op chain may keep extra
                        # precision on HW and break the exact-integer split)
                        rtmp = wk.tile([pn, NS], F32, tag=f"rt{ci}", bufs=2,
                                       name=f"rt{b}{ci}_{li}")
                        nc.vector.tensor_scalar(rtmp[:], xf[:],
                                                8388608.0, None, ALU.add)
                        nc.vector.tensor_scalar(pre[:, 0:72:2], rtmp[:],
                                                8388608.0, None, ALU.subtract)
                        nc.vector.tensor_tensor(pre[:, 1:72:2], xf[:],
                                                pre[:, 0:72:2], ALU.subtract)
                        nc.vector.memset(pre[:, 72:96], 0.0)
                        nc.tensor.transpose(trp[:, p0:p0 + pn], pre[:],
                                            Cf("ident", 0, pn, 0, pn))
                    xfif = big.tile([96, NP], FP16, tag="xfif", bufs=3,
                                    name=f"xfif{b}_{li}")
                    nc.vector.tensor_copy(xfif[:], trp[:])

                    # tents (pair pipeline, stage2 lagged by one pair)
                    def s2(g, pvt, uu, last, r0=r0, u=u, rcsbs=rcsbs):
                        hi, lo = rcsbs[g // GPT]
                        c0 = (g % GPT) * 64
                        nc.tensor.matmul(
                            fpsT[r0:r0 + 64, u * NP:(u + 1) * NP],
                            hi[0:PW, c0:c0 + 64],
                            pvt[0:PW, uu * NP:(uu + 1) * NP],
                            start=(g == 0), stop=(last and lo is None))
                        if lo is not None:
                            nc.tensor.matmul(
                                fpsT[r0:r0 + 64, u * NP:(u + 1) * NP],
                                lo[0:PW, c0:c0 + 64],
                                pvt[0:PW, uu * NP:(uu + 1) * NP],
                                start=False, stop=last)

                    pend = None
                    pi = 0
                    for g0 in range(0, G, 2):
                        npair = min(2, G - g0)
                        xfps = psA.tile([128, 2 * NP], F32, tag="xfps",
                                        name=f"xfps{b}_{li}_{g0}")
                        for uu in range(npair):
                            g = g0 + uu
                            k = (g * pack) // 16
                            v = g - k * nvar
                            nc.tensor.matmul(
                                xfps[0:PW, uu * NP:(uu + 1) * NP],
                                Ch(f"bandc{li}", 32 * k, 32, v * 128, PW),
                                xfif[32 * k:32 * k + 32, :],
                                start=True, stop=True,
                                tile_position=(32 * k, 0))
                        d1 = big.tile([128, 2 * NP], F32, tag="d1", bufs=2,
                                      name=f"d1{b}_{li}_{g0}")
                        mod = ABS_DVE_MOD[li]
                        if mod and pi % mod == 0:
                            nc.vector.tensor_scalar(
                                d1[0:PW, 0:npair * NP],
                                xfps[0:PW, 0:npair * NP],
                                Cf(f"negiota{li}", 0, PW), 0.0,
                                ALU.add, ALU.abs_max)
                        else:
                            nc.scalar.activation(d1[0:PW, 0:npair * NP],
                                                 xfps[0:PW, 0:npair * NP],
                                                 AF.Abs,
                                                 bias=Cf(f"negiota{li}", 0, PW))
                        vt = big.tile([128, 2 * NP],
                                      FP16 if GATHER == "dual" else F32,
                                      tag="vt", bufs=3,
                                      name=f"vt{b}_{li}_{g0}")
                        mm_ = MIN_POOL_MOD[li]
                        if mm_ > 0 and pi % mm_ == 0:
                            nc.gpsimd.tensor_scalar(vt[0:PW, 0:npair * NP],
                                                    d1[0:PW, 0:npair * NP],
                                                    1.0, 0.0,
                                                    ALU.subtract, ALU.min)
                        else:
                            nc.vector.tensor_scalar(vt[0:PW, 0:npair * NP],
                                                    d1[0:PW, 0:npair * NP],
                                                    1.0, 0.0,
                                                    ALU.subtract, ALU.min)
                        if pend is not None:
                            pg0, pn_, pvt = pend
                            for uu in range(pn_):
                                s2(pg0 + uu, pvt, uu, False)
                        pend = (g0, npair, vt)
                        pi += 1
                    pg0, pn_, pvt = pend
                    for uu in range(pn_):
                        g = pg0 + uu
                        s2(g, pvt, uu, g == G - 1)

                fTf = big.tile([64, 2 * NP], F32, tag="fTf", bufs=2,
                               name=f"fTf_{bp}_{li}")
                nc.scalar.activation(fTf[:], fpsT[r0:r0 + 64, :], AF.Relu,
                                     bias=Cf("b_fc"))
                for u in range(2):
                    b = bp * 2 + u
                    nc.vector.tensor_scalar(fTf[:, u * NP:(u + 1) * NP],
                                            fTf[:, u * NP:(u + 1) * NP],
                                            tokT[:, b:b + 1], None, ALU.add)
                fT16 = big.tile([64, 2 * NP], FP16, tag="fT16", bufs=2,
                                name=f"fT16_{bp}_{li}")
                nc.vector.tensor_copy(fT16[:], fTf[:])
                fT16s.append((fT16, fTf))

            # --- phase C1: q/k/v + scores + exp (item-paired) ---
            for bp in range(NB // 2):
                fT16, fTf = fT16s[bp]
                qp = psD.tile([64, 2 * NP], F32, tag="mm", name=f"qp{bp}_{li}")
                nc.tensor.matmul(qp[:], Cf("Wf_q"), fTf[:])
                qT = wk.tile([64, 2 * NP], F32, tag="qT", name=f"qT{bp}_{li}")
                nc.vector.tensor_scalar(qT[:], qp[:], 0.125, None, ALU.mult)
                kp = psD.tile([64, 2 * NP], F32, tag="mm", name=f"kp{bp}_{li}")
                nc.tensor.matmul(kp[:], Cf("Wf_k"), fTf[:])
                kT = wk.tile([64, 2 * NP], F32, tag="kT", name=f"kT{bp}_{li}")
                nc.vector.tensor_copy(kT[:], kp[:])
                for ci, (p0, pn) in enumerate(PCH):
                    vp2 = psD.tile([128, 128], F32, tag="mm",
                                   name=f"vp{bp}{ci}_{li}")
                    sp2 = psD.tile([128, 2 * NP], F32, tag="mm",
                                   name=f"sp{bp}{ci}_{li}")
                    for u in range(2):
                        nc.tensor.matmul(vp2[0:pn, u * 64:(u + 1) * 64],
                                         fTf[:, u * NP + p0:u * NP + p0 + pn],
                                         Cf("Wf_v"))
                        nc.tensor.matmul(sp2[0:pn, u * NP:(u + 1) * NP],
                                         kT[:, u * NP + p0:u * NP + p0 + pn],
                                         qT[:, u * NP:(u + 1) * NP])
                    vtl = wk.tile([pn, 128], FP16, tag=f"vn{ci}", bufs=2,
                                  name=f"vn{bp}{ci}_{li}")
                    nc.vector.tensor_copy(vtl[:], vp2[0:pn, :])
                    vn2[(bp, ci)] = vtl
                    e = wk.tile([pn, 2 * NP], FP16, tag=f"est{ci}", bufs=2,
                                name=f"est{bp}{ci}_{li}")
                    nc.scalar.activation(e[:], sp2[0:pn, :], AF.Exp)
                    est2[(bp, ci)] = e

            # --- phase C2: softmax denom + attn out + FiLM (item-paired) ---
            fHp = []
            for bp in range(NB // 2):
                rbc2 = wk.tile([64, 2 * NP], F32, tag="rbc", bufs=2,
                               name=f"rbc{bp}_{li}")
                for u in range(2):
                    b = bp * 2 + u
                    zp = psD.tile([1, NP], F32, tag="mm", name=f"zp{b}_{li}")
                    for ci, (p0, pn) in enumerate(PCH):
                        nc.tensor.matmul(zp[:], Ch("ones16", 0, pn),
                                         est2[(bp, ci)][:, u * NP:(u + 1) * NP],
                                         start=(ci == 0), stop=(ci == 1))
                    rrow = wk.tile([1, NP], F32, tag="rrow",
                                   name=f"rrow{b}_{li}")
                    nc.vector.reciprocal(rrow[:], zp[:])
                    nc.gpsimd.partition_broadcast(rbc2[:, u * NP:(u + 1) * NP],
                                                  rrow[0:1, :], channels=64)
                avp2 = psD.tile([64, 2 * NP], F32, tag="mm", name=f"av{bp}_{li}")
                for u in range(2):
                    for ci in range(2):
                        nc.tensor.matmul(
                            avp2[:, u * NP:(u + 1) * NP],
                            vn2[(bp, ci)][:, u * 64:(u + 1) * 64],
                            est2[(bp, ci)][:, u * NP:(u + 1) * NP],
                            start=(ci == 0), stop=(ci == 1))
                avsb = wk.tile([64, 2 * NP], FP16, tag="avsb",
                               name=f"av{bp}_{li}")
                nc.vector.tensor_copy(avsb[:], avp2[:])
                opp = psD.tile([64, 2 * NP], F32, tag="mm", name=f"opp{bp}_{li}")
                nc.tensor.matmul(opp[:], Ch("W_o"), avsb[:])
                t12 = wk.tile([64, 2 * NP], F32, tag="attnt",
                              name=f"t1{bp}_{li}")
                nc.vector.tensor_tensor(t12[:], opp[:], rbc2[:], ALU.mult)
                nc.vector.tensor_tensor(t12[:], fT16s[bp][1][:], t12[:],
                                        ALU.add)
                fH = big.tile([64, 2 * NP], F32, tag="fH", bufs=2,
                              name=f"fH{bp}_{li}")
                for u in range(2):
                    b = bp * 2 + u
                    nc.vector.tensor_scalar(fH[:, u * NP:(u + 1) * NP],
                                            t12[:, u * NP:(u + 1) * NP],
                                            scale1T[:, b:b + 1],
                                            shiftT[:, b:b + 1],
                                            ALU.mult, ALU.add)
                fHp.append(fH)

            pending[0] = (lambda li=li, W=W, fHp=fHp, is_last=is_last:
                          emit_heads(li, W, fHp, is_last))

        pending[0]()

    nc.compile()
    _CACHE.pop("regn", None)
    return nc


def _host_inputs(inp_slice, nwfc_l):
    m = {}
    feats = [inp_slice["feat2"], inp_slice["feat1"], inp_slice["feat0"]]
    for li, (H, W, Wp, pack) in enumerate(LEVELS):
        f = np.asarray(feats[li], np.float32)  # (NB, 64, H, W)
        y1 = np.minimum(np.arange(H) + 1, H - 1)
        top = np.zeros((64, NB, H, Wp), np.float32)
        bot = np.zeros((64, NB, H, Wp), np.float32)
        top[:, :, :, :W] = f.transpose(1, 0, 2, 3)
        bot[:, :, :, :W] = f[:, :, y1, :].transpose(1, 0, 2, 3)
        st = np.concatenate([top.reshape(64, -1), bot.reshape(64, -1)], axis=0)
        m[f"festack{li}"] = st.astype(np.float16)

    w = {k: np.asarray(v, np.float32) for k, v in inp_slice.items()
         if k.startswith(("W_", "b_"))}

    cp = np.zeros((128, F32_NCOL), np.float32)

    def put32(name, val):
        off, r, c = F32_OFFS[name]
        val = np.asarray(val, np.float32)
        if val.ndim == 1:
            val = val.reshape(-1, 1)
        cp[0:val.shape[0], off:off + val.shape[1]] = val

    for li in range(3):
        qs = Q_S[_s_of_t(li)]
        put32(f"qrep{li}", np.broadcast_to(qs[None, :], (128, NS)))
        put32(f"negiota{li}",
              -(np.arange(128, dtype=np.float32) % LEVELS[li][2]))
    put32("qfrep", np.broadcast_to(QF_R[None, :], (128, NR)))
    put32("halfpi", np.full((128, 1), math.pi / 2.0))
    put32("ident", np.eye(128))
    put32("W_t1", w["W_t1"])
    put32("b_t1", np.ascontiguousarray(w["b_t1"].reshape(2, 128).T))
    put32("W_t2a", w["W_t2"][:128]); put32("W_t2b", w["W_t2"][128:])
    put32("b_t2", np.ascontiguousarray(w["b_t2"].reshape(2, 128).T))
    put32("W_sta", w["W_st"][:128]); put32("W_stb", w["W_st"][128:])
    put32("bstS1", w["b_st"][:64] + 1.0)
    put32("bstSh", w["b_st"][64:])
    put32("W_tca", w["W_tc"][:128]); put32("W_tcb", w["W_tc"][128:])
    for nm in ["b_tc", "b_fc", "b_c1", "b_c2", "b_r1", "b_r2", "b_cls", "b_reg"]:
        put32(nm, w[nm])
    for nm in ["c1", "c2", "r1", "r2", "cls", "reg", "q", "k", "v"]:
        put32(f"Wf_{nm}", w[f"W_{nm}"])
    m["cpak"] = cp

    hpa = np.zeros((128, F16A_NCOL), np.float16)
    hpb = np.zeros((128, F16B_NCOL), np.float16)

    def put16(name, val):
        offs, hp = (F16A_OFFS, hpa) if name in F16A_OFFS else (F16B_OFFS, hpb)
        off, r, c = offs[name]
        val = np.asarray(val)
        if val.ndim == 1:
            val = val.reshape(-1, 1)
        hp[0:val.shape[0], off:off + val.shape[1]] = val.astype(np.float16)

    for li in range(3):
        put16(f"nwfc2_{li}", nwfc_l[li])
        put16(f"bandc{li}", _bandc(LEVELS[li][2], LEVELS[li][3]))
    for nm in ["W_q", "W_k", "W_v", "W_o", "W_c1", "W_c2", "W_r1", "W_r2",
               "W_cls", "W_reg"]:
        put16(nm, w[nm])
    put16("ones16", np.ones((128, 1)))
    m["hpak0"] = hpa
    m["hpak1"] = hpb

    a = np.asarray(inp_slice["inputs"], np.float32)  # (NB, NP, 3)
    a0 = np.zeros((128, NB * 3), np.float32)
    a1 = np.zeros((64, NB * 3), np.float32)
    for b in range(NB):
        a0[:, b * 3:(b + 1) * 3] = a[b, 0:128, :]
        a1[:, b * 3:(b + 1) * 3] = a[b, 128:192, :]
    m["anch0"] = a0
    m["anch1"] = a1

    half = FC // 2
    freqs = np.exp(np.arange(half, dtype=np.float32)
                   * (-math.log(10000.0) / (half - 1)))
    ang = np.asarray(inp_slice["t"]).astype(np.float32)[:, None] * freqs[None, :]
    full = np.concatenate([ang, ang + math.pi / 2.0], axis=1)
    full = np.mod(full + math.pi, 2.0 * math.pi) - math.pi
    m["sinargsT"] = np.ascontiguousarray(full.T).astype(np.float32)
    return {k: np.ascontiguousarray(np.asarray(v)) for k, v in m.items()}


def make_in_maps(inputs):
    inputs = {k: np.asarray(v) for k, v in inputs.items()}
    W_fc = np.asarray(inputs["W_fc"], np.float32)
    nwfc_l = [_nwfc2(W_fc, H) for H, W, Wp, pack in LEVELS]
    in_maps = []
    for c in range(N_CORES):
        sl = slice(c * NB, (c + 1) * NB)
        inp_slice = {k: (v[sl] if k in ("feat0", "feat1", "feat2", "inputs", "t")
                         else v) for k, v in inputs.items()}
        in_maps.append(_host_inputs(inp_slice, nwfc_l))
    return in_maps


def kernel(**inputs):
    if "prog" not in _CACHE:
        _CACHE["prog"] = _build_program()
    nc = _CACHE["prog"]
    in_maps = make_in_maps(inputs)
    res = bass_utils.run_bass_kernel_spmd(nc, in_maps,
                                          core_ids=list(range(N_CORES)))
    out = np.concatenate([res.results[c]["out"] for c in range(N_CORES)], axis=0)
    return np.ascontiguousarray(out.astype(np.float32))


# revision 51
# speedup vs baseline: 1.1792x; 1.1792x over previous
"""Trainium2 Bass kernel for nn_CLRerHead (CLRNet-style lane-detection head).

Sharding: data-parallel over batch. 32 items -> 8 cores x NB=4 items.

v3: "negative tent" matmul gather with s-packing, corner-folded RC,
exact xi+frac fp16 band-broadcast, software-pipelined tent pair loop,
item-paired attention/FiLM/heads, engine-balanced abs/min/copy placement,
host-side sample permutation enabling run-batched RC matmuls.
"""

import math
import numpy as np
from contextlib import ExitStack

import concourse.bacc as bacc
import concourse.mybir as mybir
import concourse.tile as tile
from concourse import bass_utils

dt = mybir.dt
AF = mybir.ActivationFunctionType
ALU = mybir.AluOpType

# ---------------- static problem config ----------------
IMG_W, IMG_H = 800.0, 320.0
NR, NS, NP, FC = 72, 36, 192, 64
N_STRIPS = NR - 1
ALPHA = IMG_H / IMG_W
SAMPLE_IDX = (np.linspace(0.0, 1.0, NS) * N_STRIPS).astype(np.int64)
PRIOR_FEAT_YS = np.flip(SAMPLE_IDX.astype(np.float32) / N_STRIPS).copy()
PRIOR_YS = np.linspace(1.0, 0.0, NR, dtype=np.float32)

N_CORES = 8
NB = 4
# (H, W, Wp, pack): Wp = x-padded width, pack = s-values per psum group
LEVELS = [(10, 25, 32, 4), (20, 50, 64, 2), (40, 100, 100, 1)]
PCH = [(0, 128), (128, 64)]
FP16 = dt.float16
F32 = dt.float32

Q_S = (1.0 - PRIOR_YS[SAMPLE_IDX[::-1]]).astype(np.float32)
QF_R = (1.0 - PRIOR_YS).astype(np.float32)

GPT = 8  # rc groups per psum tile (8 * 64 cols = 512 f32 = one bank)

# engine balance knobs: emit DVE-abs when pair_idx % mod == 0 (0 = never)
# NOTE: abs_max/mod are NOT valid DVE tensor_scalar ops on real HW (walrus
# ISA check) — keep ABS_DVE_MOD at 0 everywhere.
ABS_DVE_MOD = {0: 0, 1: 0, 2: 0}
# tent-min goes to Pool when pair_idx % mod == 0 (0 = always DVE)
MIN_POOL_MOD = {0: 1, 1: 2, 2: 3}
RC_ACT = {0: False, 1: False, 2: False}   # rc copy: True=ACT False=DVE None=alt
HEADS_DVE = False                          # head bias+relu on DVE instead of ACT
# gather precision: "fp16" | "dual" (hi+lo fp16, ~21-bit, 2 matmuls/group)
GATHER = "dual"


def _s_of_t(li):
    """Sample index s for transposed slot t (band-major packing)."""
    H, W, Wp, pack = LEVELS[li]
    G = NS // pack
    t = np.arange(NS)
    return (t % pack) * G + t // pack


def _level_ytab(H):
    ys = PRIOR_FEAT_YS * (H - 1)
    y0 = np.clip(np.floor(ys).astype(np.int64), 0, H - 1)
    y1 = np.minimum(y0 + 1, H - 1)
    wy1 = (ys - y0).astype(np.float32)
    wy1 = np.where(y1 == y0, 0.0, wy1).astype(np.float32)
    wy0 = (1.0 - wy1).astype(np.float32)
    return y0, y1, wy0, wy1


def _nwfc2(W_fc, H):
    """[128, NS*64] fp16: rows corner*64+ch = -wy_corner[s] * Wfc[ch*NS+s, d]."""
    _, _, wy0, wy1 = _level_ytab(H)
    out = np.zeros((128, NS * 64), np.float32)
    for s in range(NS):
        blk = W_fc[s::NS, :]  # (64ch, 64d)
        out[0:64, s * 64:(s + 1) * 64] = -wy0[s] * blk
        out[64:128, s * 64:(s + 1) * 64] = -wy1[s] * blk
    return out.astype(np.float16)


def _bandc(Wp, pack):
    """Band const [96, nvar*128] fp16 replicated at row bases 0/32/64."""
    nvar = 16 // pack
    one = np.zeros((32, nvar * 128), np.float32)
    for v in range(nvar):
        for j in range(pack):
            r = 2 * (v * pack + j)
            one[r, v * 128 + j * Wp:v * 128 + (j + 1) * Wp] = 1.0
            one[r + 1, v * 128 + j * Wp:v * 128 + (j + 1) * Wp] = 1.0
    return np.concatenate([one, one, one], axis=0).astype(np.float16)


_CACHE = {}

F32_CONSTS = [
    ("qrep0", 128, NS), ("qrep1", 128, NS), ("qrep2", 128, NS),
    ("qfrep", 128, NR), ("halfpi", 128, 1),
    ("negiota0", 128, 1), ("negiota1", 128, 1), ("negiota2", 128, 1),
    ("ident", 128, 128),
    ("W_t1", 64, 256), ("b_t1", 128, 2),
    ("W_t2a", 128, 256), ("W_t2b", 128, 256), ("b_t2", 128, 2),
    ("W_sta", 128, 128), ("W_stb", 128, 128), ("bstS1", 64, 1), ("bstSh", 64, 1),
    ("W_tca", 128, 64), ("W_tcb", 128, 64), ("b_tc", 64, 1),
    ("b_fc", 64, 1), ("b_c1", 64, 1), ("b_c2", 64, 1),
    ("b_r1", 64, 1), ("b_r2", 64, 1), ("b_cls", 2, 1), ("b_reg", 76, 1),
    # f32 head weights: the reg head feeds anchor updates whose xs
    # sensitivity is ~25x, so this chain stays in f32
    ("Wf_c1", 64, 64), ("Wf_c2", 64, 64), ("Wf_r1", 64, 64),
    ("Wf_r2", 64, 64), ("Wf_cls", 64, 2), ("Wf_reg", 64, 76),
    ("Wf_q", 64, 64), ("Wf_k", 64, 64), ("Wf_v", 64, 64),
]
# fp16 consts split into two packs: hpak0 needed first (level 2 + attention)
F16A_CONSTS = [
    ("nwfc2_0", 128, NS * 64),
    ("bandc0", 96, (16 // LEVELS[0][3]) * 128),
    ("W_q", 64, 64), ("W_k", 64, 64), ("W_v", 64, 64), ("W_o", 64, 64),
    ("W_c1", 64, 64), ("W_c2", 64, 64), ("W_r1", 64, 64), ("W_r2", 64, 64),
    ("W_cls", 64, 2), ("W_reg", 64, 76), ("ones16", 128, 1),
]
F16B_CONSTS = [
    ("nwfc2_1", 128, NS * 64), ("nwfc2_2", 128, NS * 64),
    ("bandc1", 96, (16 // LEVELS[1][3]) * 128),
    ("bandc2", 96, (16 // LEVELS[2][3]) * 128),
]


def _pack_offsets(spec):
    offs, col = {}, 0
    for name, rows, cols in spec:
        offs[name] = (col, rows, cols)
        col += cols
    return offs, col


F32_OFFS, F32_NCOL = _pack_offsets(F32_CONSTS)
F16A_OFFS, F16A_NCOL = _pack_offsets(F16A_CONSTS)
F16B_OFFS, F16B_NCOL = _pack_offsets(F16B_CONSTS)


def _build_program(num_devices=N_CORES):
    nc = bacc.Bacc("TRN2", target_bir_lowering=False, debug=False,
                   num_devices=num_devices)
    D = {}

    def din(name, shape, dtype=F32):
        D[name] = nc.dram_tensor(name, list(shape), dtype, kind="ExternalInput")

    din("cpak", (128, F32_NCOL))
    din("hpak0", (128, F16A_NCOL), FP16)
    din("hpak1", (128, F16B_NCOL), FP16)
    for li, (H, W, Wp, pack) in enumerate(LEVELS):
        din(f"festack{li}", (128, NB * H * Wp), FP16)
    din("anch0", (128, NB * 3))
    din("anch1", (64, NB * 3))
    din("sinargsT", (64, NB))

    out_t = nc.dram_tensor("out", [NB, NP, 78], F32, kind="ExternalOutput")

    with tile.TileContext(nc) as tc, ExitStack() as ex:
        cpool = ex.enter_context(tc.tile_pool(name="consts", bufs=1))
        state = ex.enter_context(tc.tile_pool(name="state", bufs=1))
        wk = ex.enter_context(tc.tile_pool(name="work", bufs=2))
        big = ex.enter_context(tc.tile_pool(name="big", bufs=2))
        psA = ex.enter_context(tc.tile_pool(name="psA", bufs=3, space="PSUM"))
        psB = ex.enter_context(tc.tile_pool(name="psB", bufs=2, space="PSUM"))
        psC = ex.enter_context(tc.tile_pool(name="psC", bufs=1, space="PSUM"))
        psD = ex.enter_context(tc.tile_pool(name="psD", bufs=2, space="PSUM"))

        cpak = cpool.tile([128, F32_NCOL], F32, tag="cpak", name="cpak")
        hpak0 = cpool.tile([128, F16A_NCOL], FP16, tag="hpak0", name="hpak0")
        hpak1 = cpool.tile([128, F16B_NCOL], FP16, tag="hpak1", name="hpak1")
        nc.sync.dma_start(cpak[:], D["cpak"].ap())

        def Cf(name, p0=0, pn=None, c0=0, cn=None):
            off, rows, cols = F32_OFFS[name]
            pn = rows if pn is None else pn
            cn = cols if cn is None else cn
            return cpak[p0:p0 + pn, off + c0:off + c0 + cn]

        def Ch(name, p0=0, pn=None, c0=0, cn=None):
            if name in F16A_OFFS:
                off, rows, cols = F16A_OFFS[name]
                pak = hpak0
            else:
                off, rows, cols = F16B_OFFS[name]
                pak = hpak1
            pn = rows if pn is None else pn
            cn = cols if cn is None else cn
            return pak[p0:p0 + pn, off + c0:off + c0 + cn]

        anch = []
        for ci, (p0, pn) in enumerate(PCH):
            a = state.tile([pn, NB * 3], F32, tag=f"anch{ci}", name=f"anch{ci}")
            nc.sync.dma_start(a[:], D[f"anch{ci}"].ap())
            anch.append(a)
        sarg = cpool.tile([64, NB], F32, tag="sarg", name="sarg")
        nc.sync.dma_start(sarg[:], D["sinargsT"].ap())
        nc.sync.dma_start(hpak0[:], D["hpak0"].ap())

        fst = []
        for li, (H, W, Wp, pack) in enumerate(LEVELS):
            t = cpool.tile([128, NB * H * Wp], FP16, tag=f"fst{li}",
                           name=f"fst{li}")
            nc.sync.dma_start(t[:], D[f"festack{li}"].ap())
            fst.append(t)
            if li == 0:
                nc.sync.dma_start(hpak1[:], D["hpak1"].ap())

        osts = {}
        for b in range(NB):
            for ci, (p0, pn) in enumerate(PCH):
                osts[(b, ci)] = state.tile([pn, 78], F32, tag=f"ost{b}_{ci}",
                                           name=f"ost{b}_{ci}")

        # ---------------- time MLP ----------------
        sinT = wk.tile([64, NB], F32, tag="tm_sin", name="sinT")
        nc.scalar.activation(sinT[:], sarg[:], AF.Sin)
        emb = []
        for m in range(2):
            p = psD.tile([128, NB], F32, tag="mm", name=f"p_emb{m}")
            nc.tensor.matmul(p[:], Cf("W_t1", 0, 64, m * 128, 128), sinT[:])
            x = state.tile([128, NB], F32, tag=f"emb{m}", name=f"emb{m}")
            nc.scalar.activation(x[:], p[:], AF.Identity,
                                 bias=Cf("b_t1", 0, 128, m, 1))
            sq = wk.tile([128, NB], F32, tag="tm_sq", name=f"sq{m}")
            nc.scalar.activation(sq[:], x[:], AF.Square)
            cu = wk.tile([128, NB], F32, tag="tm_cu", name=f"cu{m}")
            nc.vector.tensor_tensor(cu[:], sq[:], x[:], ALU.mult)
            nc.vector.tensor_scalar(cu[:], cu[:], 0.044715, None, ALU.mult)
            nc.vector.tensor_tensor(cu[:], cu[:], x[:], ALU.add)
            th = wk.tile([128, NB], F32, tag="tm_th", name=f"th{m}")
            nc.scalar.activation(th[:], cu[:], AF.Tanh,
                                 scale=float(np.sqrt(2.0 / np.pi)))
            nc.vector.tensor_scalar(th[:], th[:], 1.0, 0.5, ALU.add, ALU.mult)
            nc.vector.tensor_tensor(x[:], th[:], x[:], ALU.mult)
            emb.append(x)
        tmb = []
        for m in range(2):
            p = psD.tile([128, NB], F32, tag="mm", name=f"p_tmb{m}")
            for k in range(2):
                wt2 = Cf("W_t2a" if k == 0 else "W_t2b", 0, 128, m * 128, 128)
                nc.tensor.matmul(p[:], wt2, emb[k][:],
                                 start=(k == 0), stop=(k == 1))
            x = state.tile([128, NB], F32, tag=f"tmb{m}", name=f"tmb{m}")
            nc.scalar.activation(x[:], p[:], AF.Identity,
                                 bias=Cf("b_t2", 0, 128, m, 1))
            tmb.append(x)
        sil = []
        for m in range(2):
            s = wk.tile([128, NB], F32, tag=f"tm_sil{m}", name=f"sil{m}")
            nc.scalar.activation(s[:], tmb[m][:], AF.Sigmoid)
            nc.vector.tensor_tensor(s[:], s[:], tmb[m][:], ALU.mult)
            sil.append(s)
        scale1T = state.tile([64, NB], F32, tag="scale1T", name="scale1T")
        shiftT = state.tile([64, NB], F32, tag="shiftT", name="shiftT")
        for j, (dst, bias) in enumerate([(scale1T, "bstS1"), (shiftT, "bstSh")]):
            p = psD.tile([64, NB], F32, tag="mm", name=f"p_ss{j}")
            for k in range(2):
                wst = Cf("W_sta" if k == 0 else "W_stb", 0, 128, j * 64, 64)
                nc.tensor.matmul(p[:], wst, sil[k][:],
                                 start=(k == 0), stop=(k == 1))
            nc.scalar.activation(dst[:], p[:], AF.Identity, bias=Cf(bias))
        tokT = state.tile([64, NB], F32, tag="tokT", name="tokT")
        ptk = psD.tile([64, NB], F32, tag="mm", name="p_tok")
        for k in range(2):
            wtc = Cf("W_tca" if k == 0 else "W_tcb")
            nc.tensor.matmul(ptk[:], wtc, tmb[k][:], start=(k == 0), stop=(k == 1))
        nc.scalar.activation(tokT[:], ptk[:], AF.Identity, bias=Cf("b_tc"))

        # ---------------- per-level helpers ----------------
        def gen_ab(li, W, scaleW, c0=0, nb=NB):
            """Batched trig across items [c0, c0+nb): per-chunk (aC,bC,base,g)."""
            res = []
            for ci, (p0, pn) in enumerate(PCH):
                A = anch[ci]
                lo, hi = c0 * 3, (c0 + nb) * 3
                sn = wk.tile([pn, nb], F32, tag=f"sn{ci}", name=f"sn{ci}_{li}_{c0}")
                cs = wk.tile([pn, nb], F32, tag=f"cs{ci}", name=f"cs{ci}_{li}_{c0}")
                nc.scalar.activation(sn[:], A[:, lo + 2:hi:3], AF.Sin,
                                     scale=math.pi)
                nc.scalar.activation(cs[:], A[:, lo + 2:hi:3], AF.Sin,
                                     scale=-math.pi, bias=Cf("halfpi", 0, pn))
                g = wk.tile([pn, nb], F32, tag=f"g{ci}", name=f"g{ci}_{li}_{c0}")
                nc.vector.reciprocal(g[:], sn[:])
                nc.vector.tensor_tensor(g[:], cs[:], g[:], ALU.mult)
                nc.vector.tensor_scalar(g[:], g[:], 1000.0, -1000.0,
                                        ALU.min, ALU.max)
                nc.vector.tensor_scalar(g[:], g[:], ALPHA, None, ALU.mult)
                base = wk.tile([pn, nb], F32, tag=f"bs{ci}", name=f"bs{ci}_{li}_{c0}")
                nc.vector.tensor_tensor(base[:], A[:, lo:hi:3], g[:], ALU.mult)
                nc.vector.tensor_tensor(base[:], A[:, lo + 1:hi:3], base[:],
                                        ALU.subtract)
                aC = wk.tile([pn, nb], F32, tag=f"aC{ci}", name=f"aC{ci}_{li}_{c0}")
                bC = wk.tile([pn, nb], F32, tag=f"bC{ci}", name=f"bC{ci}_{li}_{c0}")
                nc.vector.tensor_scalar(aC[:], base[:], scaleW, None, ALU.mult)
                nc.vector.tensor_scalar(bC[:], g[:], scaleW, None, ALU.mult)
                res.append((aC, bC, base, g))
            return res

        def head_mm(li, wname, bias, src, relu=True, out_p=64, tag="hd"):
            # f32 chain: moving src f32 + f32 stationary weights
            p = psD.tile([128, 2 * NP], F32, tag="mm", name=f"p{tag}_{li}")
            nc.tensor.matmul(p[0:out_p, :], Cf(wname), src[:])
            o = wk.tile([out_p, 2 * NP], F32, tag=f"hd_{tag}",
                        name=f"{tag}o_{li}")
            if HEADS_DVE:
                if relu:
                    nc.vector.tensor_scalar(o[:], p[0:out_p, :],
                                            Cf(bias, 0, out_p), 0.0,
                                            ALU.add, ALU.max)
                else:
                    nc.vector.tensor_scalar(o[:], p[0:out_p, :],
                                            Cf(bias, 0, out_p), None, ALU.add)
            else:
                nc.scalar.activation(o[:], p[0:out_p, :],
                                     AF.Relu if relu else AF.Identity,
                                     bias=Cf(bias, 0, out_p))
            return o

        def emit_heads(li, W, fHp, is_last):
            for bp in range(NB // 2):
                fH = fHp[bp]
                r1 = head_mm(li, "Wf_r1", "b_r1", fH, tag=f"r1{bp}")
                r2 = head_mm(li, "Wf_r2", "b_r2", r1, tag=f"r2{bp}")
                regT = head_mm(li, "Wf_reg", "b_reg", r2, relu=False, out_p=76,
                               tag=f"rg{bp}")
                clsT = None
                if is_last:
                    c1 = head_mm(li, "Wf_c1", "b_c1", fH, tag=f"c1{bp}")
                    c2 = head_mm(li, "Wf_c2", "b_c2", c1, tag=f"c2{bp}")
                    clsT = head_mm(li, "Wf_cls", "b_cls", c2, relu=False,
                                   out_p=2, tag=f"cl{bp}")
                for ci, (p0, pn) in enumerate(PCH):
                    pt2 = psD.tile([128, 2 * 76], F32, tag="mm",
                                   name=f"p_rt{bp}{ci}_{li}")
                    for u in range(2):
                        nc.tensor.transpose(
                            pt2[0:pn, u * 76:u * 76 + 76],
                            regT[:, u * NP + p0:u * NP + p0 + pn],
                            Cf("ident", 0, 76, 0, 76))
                    rn2 = state.tile([pn, 2 * 76], F32, tag=f"regn{bp}_{ci}",
                                     name=f"regn{bp}{ci}_{li}")
                    nc.vector.tensor_copy(rn2[:], pt2[0:pn, :])
                    A = anch[ci]
                    for u in range(2):
                        b = bp * 2 + u
                        nc.vector.tensor_tensor(A[:, b * 3:(b + 1) * 3],
                                                A[:, b * 3:(b + 1) * 3],
                                                rn2[:, u * 76:u * 76 + 3],
                                                ALU.add)
                        if is_last:
                            ost = osts[(b, ci)]
                            nc.vector.tensor_copy(ost[:, 2:5],
                                                  A[:, b * 3:(b + 1) * 3])
                            nc.vector.tensor_copy(ost[:, 5:6],
                                                  rn2[:, u * 76 + 3:u * 76 + 4])
                            _CACHE.setdefault("regn", {})[(b, ci)] = (rn2, u)
                    if is_last:
                        ptc = psD.tile([128, 4], F32, tag="mm",
                                       name=f"p_ct{bp}{ci}")
                        for u in range(2):
                            nc.tensor.transpose(
                                ptc[0:pn, u * 2:u * 2 + 2],
                                clsT[:, u * NP + p0:u * NP + p0 + pn],
                                Cf("ident", 0, 2, 0, 2))
                        for u in range(2):
                            b = bp * 2 + u
                            nc.vector.tensor_copy(osts[(b, ci)][:, 0:2],
                                                  ptc[0:pn, u * 2:u * 2 + 2])
                if is_last:
                    abf = gen_ab(li, W, 1.0, c0=bp * 2, nb=2)
                    for u in range(2):
                        b = bp * 2 + u
                        for ci, (p0, pn) in enumerate(PCH):
                            _, _, base, g = abf[ci]
                            rn2, uu = _CACHE["regn"][(b, ci)]
                            ost = osts[(b, ci)]
                            nc.vector.tensor_scalar(ost[:, 6:78],
                                                    Cf("qfrep", 0, pn),
                                                    g[:, u:u + 1],
                                                    base[:, u:u + 1],
                                                    ALU.mult, ALU.add)
                            nc.vector.tensor_tensor(
                                ost[:, 6:78], ost[:, 6:78],
                                rn2[:, uu * 76 + 4:uu * 76 + 76], ALU.add)
                            nc.sync.dma_start(out_t.ap()[b, p0:p0 + pn, :],
                                              ost[:])

        # ---------------- main loop ----------------
        pending = [None]
        for li, (H, W, Wp, pack) in enumerate(LEVELS):
            G = NS // pack
            nvar = 16 // pack
            is_last = li == len(LEVELS) - 1
            fstL = fst[li]
            ntile = (G + GPT - 1) // GPT
            y0t, _, _, _ = _level_ytab(H)
            PW = pack * Wp
            npairs = (G + 1) // 2

            # --- RC for ALL items first: independent of anchors, fills the
            # level-boundary pipeline bubble ---
            rcsbs_all = {}
            rci = 0
            for b in range(NB):
                for t in range(ntile):
                    g_lo = t * GPT
                    g_hi = min(G, g_lo + GPT)
                    rcp = psB.tile([128, 512], F32, tag="rc",
                                   name=f"rc{b}_{li}_{t}")
                    for j in range(pack):
                        g = g_lo
                        while g < g_hi:
                            s = j * G + g
                            y = int(y0t[s])
                            glen = 1
                            while (g + glen < g_hi
                                   and int(y0t[s + glen]) == y):
                                glen += 1
                            nc.tensor.matmul(
                                rcp[j * Wp:(j + 1) * Wp,
                                    (g - g_lo) * 64:(g - g_lo + glen) * 64],
                                fstL[:, (b * H + y) * Wp:
                                     (b * H + y + 1) * Wp],
                                Ch(f"nwfc2_{li}", 0, 128, s * 64, glen * 64),
                                start=True, stop=True,
                                tile_position=(0, j * Wp))
                            g += glen
                    ng = g_hi - g_lo
                    rcsb = big.tile([128, 512],
                    FP16 if GATHER == "dual" else F32,
                    tag="rcsb", bufs=22,
                                    name=f"rcsb{b}_{li}_{t}")
                    use_act = RC_ACT[li]
                    if use_act is None:
                        use_act = rci % 2 == 0
                    if use_act:
                        nc.scalar.activation(rcsb[0:PW, 0:ng * 64],
                                             rcp[0:PW, 0:ng * 64], AF.Copy)
                    else:
                        nc.vector.tensor_copy(rcsb[0:PW, 0:ng * 64],
                                              rcp[0:PW, 0:ng * 64])
                    rcsl = None
                    if GATHER == "dual":
                        rcsl = big.tile([128, 512], FP16, tag="rcsl", bufs=22,
                                        name=f"rcsl{b}_{li}_{t}")
                        nc.vector.tensor_tensor(rcsl[0:PW, 0:ng * 64],
                                                rcp[0:PW, 0:ng * 64],
                                                rcsb[0:PW, 0:ng * 64],
                                                ALU.subtract)
                    rci += 1
                    rcsbs_all.setdefault(b, []).append((rcsb, rcsl))

            # previous level's heads (anchor updates) overlap this RC block
            if pending[0] is not None:
                pending[0]()
                pending[0] = None

            ab = gen_ab(li, W, float(W - 1))

            # --- phases A+B per item-pair (single shared fps bank:
            # bp0 at partitions 0:64, bp1 at 64:128) ---
            fpsT = psC.tile([128, 2 * NP], F32, tag="fps", name=f"fpsT_{li}")
            fT16s = []
            est2 = {}
            vn2 = {}
            for bp in range(NB // 2):
                r0 = 64 * (bp % 2)
                for u in range(2):
                    b = bp * 2 + u
                    rcsbs = rcsbs_all[b]
                    # A: xf -> xi/frac interleaved -> transpose -> xfif fp16
                    trp = psD.tile([96, NP], F32, tag="mm", name=f"trp{b}_{li}")
                    for ci, (p0, pn) in enumerate(PCH):
                        aC, bC, _, _ = ab[ci]
                        pre = wk.tile([pn, 96], F32, tag=f"pre{ci}", bufs=2,
                                      name=f"pre{b}{ci}_{li}")
                        xf = wk.tile([pn, NS], F32, tag=f"xf{ci}", bufs=2,
                                     name=f"xf{b}{ci}_{li}")
                        nc.vector.tensor_scalar(xf[:], Cf(f"qrep{li}", 0, pn),
                                                bC[:, b:b + 1], aC[:, b:b + 1],
                                                ALU.mult, ALU.add)
                        nc.vector.tensor_scalar(xf[:], xf[:], float(W + 1),
                                                -2.0, ALU.min, ALU.max)
                        # xi = round(xf) via f32 2^23 add/sub; two separate
                        # instructions so the intermediate is rounded to f32
                        # in SBUF (a fused two-op chain may keep extra
                        # precision on HW and break the exact-integer split)
                        rtmp = wk.tile([pn, NS], F32, tag=f"rt{ci}", bufs=2,
                                       name=f"rt{b}{ci}_{li}")
                        nc.vector.tensor_scalar(rtmp[:], xf[:],
                                                8388608.0, None, ALU.add)
                        nc.vector.tensor_scalar(pre[:, 0:72:2], rtmp[:],
                                                8388608.0, None, ALU.subtract)
                        nc.vector.tensor_tensor(pre[:, 1:72:2], xf[:],
                                                pre[:, 0:72:2], ALU.subtract)
                        nc.vector.memset(pre[:, 72:96], 0.0)
                        nc.tensor.transpose(trp[:, p0:p0 + pn], pre[:],
                                            Cf("ident", 0, pn, 0, pn))
                    xfif = big.tile([96, NP], FP16, tag="xfif", bufs=3,
                                    name=f"xfif{b}_{li}")
                    nc.vector.tensor_copy(xfif[:], trp[:])

                    # tents (pair pipeline, stage2 lagged by one pair)
                    def s2(g, pvt, uu, last, r0=r0, u=u, rcsbs=rcsbs):
                        hi, lo = rcsbs[g // GPT]
                        c0 = (g % GPT) * 64
                        nc.tensor.matmul(
                            fpsT[r0:r0 + 64, u * NP:(u + 1) * NP],
                            hi[0:PW, c0:c0 + 64],
                            pvt[0:PW, uu * NP:(uu + 1) * NP],
                            start=(g == 0), stop=(last and lo is None))
                        if lo is not None:
                            nc.tensor.matmul(
                                fpsT[r0:r0 + 64, u * NP:(u + 1) * NP],
                                lo[0:PW, c0:c0 + 64],
                                pvt[0:PW, uu * NP:(uu + 1) * NP],
                                start=False, stop=last)

                    pend = None
                    pi = 0
                    for g0 in range(0, G, 2):
                        npair = min(2, G - g0)
                        xfps = psA.tile([128, 2 * NP], F32, tag="xfps",
                                        name=f"xfps{b}_{li}_{g0}")
                        for uu in range(npair):
                            g = g0 + uu
                            k = (g * pack) // 16
                            v = g - k * nvar
                            nc.tensor.matmul(
                                xfps[0:PW, uu * NP:(uu + 1) * NP],
                                Ch(f"bandc{li}", 32 * k, 32, v * 128, PW),
                                xfif[32 * k:32 * k + 32, :],
                                start=True, stop=True,
                                tile_position=(32 * k, 0))
                        d1 = big.tile([128, 2 * NP], F32, tag="d1", bufs=3,
                                      name=f"d1{b}_{li}_{g0}")
                        mod = ABS_DVE_MOD[li]
                        if mod and pi % mod == 0:
                            nc.vector.tensor_scalar(
                                d1[0:PW, 0:npair * NP],
                                xfps[0:PW, 0:npair * NP],
                                Cf(f"negiota{li}", 0, PW), 0.0,
                                ALU.add, ALU.abs_max)
                        else:
                            nc.scalar.activation(d1[0:PW, 0:npair * NP],
                                                 xfps[0:PW, 0:npair * NP],
                                                 AF.Abs,
                                                 bias=Cf(f"negiota{li}", 0, PW))
                        vt = big.tile([128, 2 * NP],
                                      FP16 if GATHER == "dual" else F32,
                                      tag="vt", bufs=4,
                                      name=f"vt{b}_{li}_{g0}")
                        mm_ = MIN_POOL_MOD[li]
                        if mm_ > 0 and pi % mm_ == 0:
                            nc.gpsimd.tensor_scalar(vt[0:PW, 0:npair * NP],
                                                    d1[0:PW, 0:npair * NP],
                                                    1.0, 0.0,
                                                    ALU.subtract, ALU.min)
                        else:
                            nc.vector.tensor_scalar(vt[0:PW, 0:npair * NP],
                                                    d1[0:PW, 0:npair * NP],
                                                    1.0, 0.0,
                                                    ALU.subtract, ALU.min)
                        if pend is not None:
                            pg0, pn_, pvt = pend
                            for uu in range(pn_):
                                s2(pg0 + uu, pvt, uu, False)
                        pend = (g0, npair, vt)
                        pi += 1
                    pg0, pn_, pvt = pend
                    for uu in range(pn_):
                        g = pg0 + uu
                        s2(g, pvt, uu, g == G - 1)

                fTf = big.tile([64, 2 * NP], F32, tag="fTf", bufs=2,
                               name=f"fTf_{bp}_{li}")
                nc.scalar.activation(fTf[:], fpsT[r0:r0 + 64, :], AF.Relu,
                                     bias=Cf("b_fc"))
                for u in range(2):
                    b = bp * 2 + u
                    nc.vector.tensor_scalar(fTf[:, u * NP:(u + 1) * NP],
                                            fTf[:, u * NP:(u + 1) * NP],
                                            tokT[:, b:b + 1], None, ALU.add)
                fT16 = big.tile([64, 2 * NP], FP16, tag="fT16", bufs=2,
                                name=f"fT16_{bp}_{li}")
                nc.vector.tensor_copy(fT16[:], fTf[:])
                fT16s.append((fT16, fTf))

            # --- phase C1: q/k/v + scores + exp (item-paired) ---
            for bp in range(NB // 2):
                fT16, fTf = fT16s[bp]
                qp = psD.tile([64, 2 * NP], F32, tag="mm", name=f"qp{bp}_{li}")
                nc.tensor.matmul(qp[:], Cf("Wf_q"), fTf[:])
                qT = wk.tile([64, 2 * NP], F32, tag="qT", name=f"qT{bp}_{li}")
                nc.vector.tensor_scalar(qT[:], qp[:], 0.125, None, ALU.mult)
                kp = psD.tile([64, 2 * NP], F32, tag="mm", name=f"kp{bp}_{li}")
                nc.tensor.matmul(kp[:], Cf("Wf_k"), fTf[:])
                kT = wk.tile([64, 2 * NP], F32, tag="kT", name=f"kT{bp}_{li}")
                nc.vector.tensor_copy(kT[:], kp[:])
                for ci, (p0, pn) in enumerate(PCH):
                    vp2 = psD.tile([128, 128], F32, tag="mm",
                                   name=f"vp{bp}{ci}_{li}")
                    sp2 = psD.tile([128, 2 * NP], F32, tag="mm",
                                   name=f"sp{bp}{ci}_{li}")
                    for u in range(2):
                        nc.tensor.matmul(vp2[0:pn, u * 64:(u + 1) * 64],
                                         fTf[:, u * NP + p0:u * NP + p0 + pn],
                                         Cf("Wf_v"))
                        nc.tensor.matmul(sp2[0:pn, u * NP:(u + 1) * NP],
                                         kT[:, u * NP + p0:u * NP + p0 + pn],
                                         qT[:, u * NP:(u + 1) * NP])
                    vtl = wk.tile([pn, 128], FP16, tag=f"vn{ci}", bufs=2,
                                  name=f"vn{bp}{ci}_{li}")
                    nc.vector.tensor_copy(vtl[:], vp2[0:pn, :])
                    vn2[(bp, ci)] = vtl
                    e = wk.tile([pn, 2 * NP], FP16, tag=f"est{ci}", bufs=2,
                                name=f"est{bp}{ci}_{li}")
                    nc.scalar.activation(e[:], sp2[0:pn, :], AF.Exp)
                    est2[(bp, ci)] = e

            # --- phase C2: softmax denom + attn out + FiLM (item-paired) ---
            fHp = []
            for bp in range(NB // 2):
                rbc2 = wk.tile([64, 2 * NP], F32, tag="rbc", bufs=2,
                               name=f"rbc{bp}_{li}")
                for u in range(2):
                    b = bp * 2 + u
                    zp = psD.tile([1, NP], F32, tag="mm", name=f"zp{b}_{li}")
                    for ci, (p0, pn) in enumerate(PCH):
                        nc.tensor.matmul(zp[:], Ch("ones16", 0, pn),
                                         est2[(bp, ci)][:, u * NP:(u + 1) * NP],
                                         start=(ci == 0), stop=(ci == 1))
                    rrow = wk.tile([1, NP], F32, tag="rrow",
                                   name=f"rrow{b}_{li}")
                    nc.vector.reciprocal(rrow[:], zp[:])
                    nc.gpsimd.partition_broadcast(rbc2[:, u * NP:(u + 1) * NP],
                                                  rrow[0:1, :], channels=64)
                avp2 = psD.tile([64, 2 * NP], F32, tag="mm", name=f"av{bp}_{li}")
                for u in range(2):
                    for ci in range(2):
                        nc.tensor.matmul(
                            avp2[:, u * NP:(u + 1) * NP],
                            vn2[(bp, ci)][:, u * 64:(u + 1) * 64],
                            est2[(bp, ci)][:, u * NP:(u + 1) * NP],
                            start=(ci == 0), stop=(ci == 1))
                avsb = wk.tile([64, 2 * NP], FP16, tag="avsb",
                               name=f"av{bp}_{li}")
                nc.vector.tensor_copy(avsb[:], avp2[:])
                opp = psD.tile([64, 2 * NP], F32, tag="mm", name=f"opp{bp}_{li}")
                nc.tensor.matmul(opp[:], Ch("W_o"), avsb[:])
                t12 = wk.tile([64, 2 * NP], F32, tag="attnt",
                              name=f"t1{bp}_{li}")
                nc.vector.tensor_tensor(t12[:], opp[:], rbc2[:], ALU.mult)
                nc.vector.tensor_tensor(t12[:], fT16s[bp][1][:], t12[:],
                                        ALU.add)
                fH = big.tile([64, 2 * NP], F32, tag="fH", bufs=2,
                              name=f"fH{bp}_{li}")
                for u in range(2):
                    b = bp * 2 + u
                    nc.vector.tensor_scalar(fH[:, u * NP:(u + 1) * NP],
                                            t12[:, u * NP:(u + 1) * NP],
                                            scale1T[:, b:b + 1],
                                            shiftT[:, b:b + 1],
                                            ALU.mult, ALU.add)
                fHp.append(fH)

            pending[0] = (lambda li=li, W=W, fHp=fHp, is_last=is_last:
                          emit_heads(li, W, fHp, is_last))

        pending[0]()

    nc.compile()
    _CACHE.pop("regn", None)
    return nc


def _host_inputs(inp_slice, nwfc_l):
    m = {}
    feats = [inp_slice["feat2"], inp_slice["feat1"], inp_slice["feat0"]]
    for li, (H, W, Wp, pack) in enumerate(LEVELS):
        f = np.asarray(feats[li], np.float32)  # (NB, 64, H, W)
        y1 = np.minimum(np.arange(H) + 1, H - 1)
        top = np.zeros((64, NB, H, Wp), np.float32)
        bot = np.zeros((64, NB, H, Wp), np.float32)
        top[:, :, :, :W] = f.transpose(1, 0, 2, 3)
        bot[:, :, :, :W] = f[:, :, y1, :].transpose(1, 0, 2, 3)
        st = np.concatenate([top.reshape(64, -1), bot.reshape(64, -1)], axis=0)
        m[f"festack{li}"] = st.astype(np.float16)

    w = {k: np.asarray(v, np.float32) for k, v in inp_slice.items()
         if k.startswith(("W_", "b_"))}

    cp = np.zeros((128, F32_NCOL), np.float32)

    def put32(name, val):
        off, r, c = F32_OFFS[name]
        val = np.asarray(val, np.float32)
        if val.ndim == 1:
            val = val.reshape(-1, 1)
        cp[0:val.shape[0], off:off + val.shape[1]] = val

    for li in range(3):
        qs = Q_S[_s_of_t(li)]
        put32(f"qrep{li}", np.broadcast_to(qs[None, :], (128, NS)))
        put32(f"negiota{li}",
              -(np.arange(128, dtype=np.float32) % LEVELS[li][2]))
    put32("qfrep", np.broadcast_to(QF_R[None, :], (128, NR)))
    put32("halfpi", np.full((128, 1), math.pi / 2.0))
    put32("ident", np.eye(128))
    put32("W_t1", w["W_t1"])
    put32("b_t1", np.ascontiguousarray(w["b_t1"].reshape(2, 128).T))
    put32("W_t2a", w["W_t2"][:128]); put32("W_t2b", w["W_t2"][128:])
    put32("b_t2", np.ascontiguousarray(w["b_t2"].reshape(2, 128).T))
    put32("W_sta", w["W_st"][:128]); put32("W_stb", w["W_st"][128:])
    put32("bstS1", w["b_st"][:64] + 1.0)
    put32("bstSh", w["b_st"][64:])
    put32("W_tca", w["W_tc"][:128]); put32("W_tcb", w["W_tc"][128:])
    for nm in ["b_tc", "b_fc", "b_c1", "b_c2", "b_r1", "b_r2", "b_cls", "b_reg"]:
        put32(nm, w[nm])
    for nm in ["c1", "c2", "r1", "r2", "cls", "reg", "q", "k", "v"]:
        put32(f"Wf_{nm}", w[f"W_{nm}"])
    m["cpak"] = cp

    hpa = np.zeros((128, F16A_NCOL), np.float16)
    hpb = np.zeros((128, F16B_NCOL), np.float16)

    def put16(name, val):
        offs, hp = (F16A_OFFS, hpa) if name in F16A_OFFS else (F16B_OFFS, hpb)
        off, r, c = offs[name]
        val = np.asarray(val)
        if val.ndim == 1:
            val = val.reshape(-1, 1)
        hp[0:val.shape[0], off:off + val.shape[1]] = val.astype(np.float16)

    for li in range(3):
        put16(f"nwfc2_{li}", nwfc_l[li])
        put16(f"bandc{li}", _bandc(LEVELS[li][2], LEVELS[li][3]))
    for nm in ["W_q", "W_k", "W_v", "W_o", "W_c1", "W_c2", "W_r1", "W_r2",
               "W_cls", "W_reg"]:
        put16(nm, w[nm])
    put16("ones16", np.ones((128, 1)))
    m["hpak0"] = hpa
    m["hpak1"] = hpb

    a = np.asarray(inp_slice["inputs"], np.float32)  # (NB, NP, 3)
    a0 = np.zeros((128, NB * 3), np.float32)
    a1 = np.zeros((64, NB * 3), np.float32)
    for b in range(NB):
        a0[:, b * 3:(b + 1) * 3] = a[b, 0:128, :]
        a1[:, b * 3:(b + 1) * 3] = a[b, 128:192, :]
    m["anch0"] = a0
    m["anch1"] = a1

    half = FC // 2
    freqs = np.exp(np.arange(half, dtype=np.float32)
                   * (-math.log(10000.0) / (half - 1)))
    ang = np.asarray(inp_slice["t"]).astype(np.float32)[:, None] * freqs[None, :]
    full = np.concatenate([ang, ang + math.pi / 2.0], axis=1)
    full = np.mod(full + math.pi, 2.0 * math.pi) - math.pi
    m["sinargsT"] = np.ascontiguousarray(full.T).astype(np.float32)
    return {k: np.ascontiguousarray(np.asarray(v)) for k, v in m.items()}


def make_in_maps(inputs):
    inputs = {k: np.asarray(v) for k, v in inputs.items()}
    W_fc = np.asarray(inputs["W_fc"], np.float32)
    nwfc_l = [_nwfc2(W_fc, H) for H, W, Wp, pack in LEVELS]
    in_maps = []
    for c in range(N_CORES):
        sl = slice(c * NB, (c + 1) * NB)
        inp_slice = {k: (v[sl] if k in ("feat0", "feat1", "feat2", "inputs", "t")
                         else v) for k, v in inputs.items()}
        in_maps.append(_host_inputs(inp_slice, nwfc_l))
    return in_maps


def kernel(**inputs):
    if "prog" not in _CACHE:
        _CACHE["prog"] = _build_program()
    nc = _CACHE["prog"]
    in_maps = make_in_maps(inputs)
    res = bass_utils.run_bass_kernel_spmd(nc, in_maps,
                                          core_ids=list(range(N_CORES)))
    out = np.concatenate([res.results[c]["out"] for c in range(N_CORES)], axis=0)
    return np.ascontiguousarray(out.astype(np.float32))


# revision 52
# speedup vs baseline: 1.2073x; 1.0238x over previous
"""Trainium2 Bass kernel for nn_CLRerHead (CLRNet-style lane-detection head).

Sharding: data-parallel over batch. 32 items -> 8 cores x NB=4 items.

v3: "negative tent" matmul gather with s-packing, corner-folded RC,
exact xi+frac fp16 band-broadcast, software-pipelined tent pair loop,
item-paired attention/FiLM/heads, engine-balanced abs/min/copy placement,
host-side sample permutation enabling run-batched RC matmuls.
"""

import math
import numpy as np
from contextlib import ExitStack

import concourse.bacc as bacc
import concourse.mybir as mybir
import concourse.tile as tile
from concourse import bass_utils

dt = mybir.dt
AF = mybir.ActivationFunctionType
ALU = mybir.AluOpType

# ---------------- static problem config ----------------
IMG_W, IMG_H = 800.0, 320.0
NR, NS, NP, FC = 72, 36, 192, 64
N_STRIPS = NR - 1
ALPHA = IMG_H / IMG_W
SAMPLE_IDX = (np.linspace(0.0, 1.0, NS) * N_STRIPS).astype(np.int64)
PRIOR_FEAT_YS = np.flip(SAMPLE_IDX.astype(np.float32) / N_STRIPS).copy()
PRIOR_YS = np.linspace(1.0, 0.0, NR, dtype=np.float32)

N_CORES = 8
NB = 4
# (H, W, Wp, pack): Wp = x-padded width, pack = s-values per psum group
LEVELS = [(10, 25, 32, 4), (20, 50, 64, 2), (40, 100, 100, 1)]
PCH = [(0, 128), (128, 64)]
FP16 = dt.float16
F32 = dt.float32

Q_S = (1.0 - PRIOR_YS[SAMPLE_IDX[::-1]]).astype(np.float32)
QF_R = (1.0 - PRIOR_YS).astype(np.float32)

GPT = 8  # rc groups per psum tile (8 * 64 cols = 512 f32 = one bank)

# engine balance knobs: emit DVE-abs when pair_idx % mod == 0 (0 = never)
# NOTE: abs_max/mod are NOT valid DVE tensor_scalar ops on real HW (walrus
# ISA check) — keep ABS_DVE_MOD at 0 everywhere.
ABS_DVE_MOD = {0: 0, 1: 0, 2: 0}
# tent-min goes to Pool when pair_idx % mod == 0 (0 = always DVE)
MIN_POOL_MOD = {0: 1, 1: 2, 2: 3}
RC_ACT = {0: True, 1: True, 2: None}   # rc copy: True=ACT False=DVE None=alt
HEADS_DVE = False                          # head bias+relu on DVE instead of ACT
# gather precision: "fp16" | "dual" (hi+lo fp16, ~21-bit, 2 matmuls/group)
GATHER = "dual"


def _s_of_t(li):
    """Sample index s for transposed slot t (band-major packing)."""
    H, W, Wp, pack = LEVELS[li]
    G = NS // pack
    t = np.arange(NS)
    return (t % pack) * G + t // pack


def _level_ytab(H):
    ys = PRIOR_FEAT_YS * (H - 1)
    y0 = np.clip(np.floor(ys).astype(np.int64), 0, H - 1)
    y1 = np.minimum(y0 + 1, H - 1)
    wy1 = (ys - y0).astype(np.float32)
    wy1 = np.where(y1 == y0, 0.0, wy1).astype(np.float32)
    wy0 = (1.0 - wy1).astype(np.float32)
    return y0, y1, wy0, wy1


def _nwfc2(W_fc, H):
    """[128, NS*64] fp16: rows corner*64+ch = -wy_corner[s] * Wfc[ch*NS+s, d]."""
    _, _, wy0, wy1 = _level_ytab(H)
    out = np.zeros((128, NS * 64), np.float32)
    for s in range(NS):
        blk = W_fc[s::NS, :]  # (64ch, 64d)
        out[0:64, s * 64:(s + 1) * 64] = -wy0[s] * blk
        out[64:128, s * 64:(s + 1) * 64] = -wy1[s] * blk
    return out.astype(np.float16)


def _bandc(Wp, pack):
    """Band const [96, nvar*128] fp16 replicated at row bases 0/32/64."""
    nvar = 16 // pack
    one = np.zeros((32, nvar * 128), np.float32)
    for v in range(nvar):
        for j in range(pack):
            r = 2 * (v * pack + j)
            one[r, v * 128 + j * Wp:v * 128 + (j + 1) * Wp] = 1.0
            one[r + 1, v * 128 + j * Wp:v * 128 + (j + 1) * Wp] = 1.0
    return np.concatenate([one, one, one], axis=0).astype(np.float16)


_CACHE = {}

F32_CONSTS = [
    ("qrep0", 128, NS), ("qrep1", 128, NS), ("qrep2", 128, NS),
    ("qfrep", 128, NR), ("halfpi", 128, 1),
    ("negiota0", 128, 1), ("negiota1", 128, 1), ("negiota2", 128, 1),
    ("ident", 128, 128),
    ("W_t1", 64, 256), ("b_t1", 128, 2),
    ("W_t2a", 128, 256), ("W_t2b", 128, 256), ("b_t2", 128, 2),
    ("W_sta", 128, 128), ("W_stb", 128, 128), ("bstS1", 64, 1), ("bstSh", 64, 1),
    ("W_tca", 128, 64), ("W_tcb", 128, 64), ("b_tc", 64, 1),
    ("b_fc", 64, 1), ("b_c1", 64, 1), ("b_c2", 64, 1),
    ("b_r1", 64, 1), ("b_r2", 64, 1), ("b_cls", 2, 1), ("b_reg", 76, 1),
    # f32 head weights: the reg head feeds anchor updates whose xs
    # sensitivity is ~25x, so this chain stays in f32
    ("Wf_c1", 64, 64), ("Wf_c2", 64, 64), ("Wf_r1", 64, 64),
    ("Wf_r2", 64, 64), ("Wf_cls", 64, 2), ("Wf_reg", 64, 76),
    ("Wf_q", 64, 64), ("Wf_k", 64, 64), ("Wf_v", 64, 64),
]
# fp16 consts split into two packs: hpak0 needed first (level 2 + attention)
F16A_CONSTS = [
    ("nwfc2_0", 128, NS * 64),
    ("bandc0", 96, (16 // LEVELS[0][3]) * 128),
    ("W_q", 64, 64), ("W_k", 64, 64), ("W_v", 64, 64), ("W_o", 64, 64),
    ("W_c1", 64, 64), ("W_c2", 64, 64), ("W_r1", 64, 64), ("W_r2", 64, 64),
    ("W_cls", 64, 2), ("W_reg", 64, 76), ("ones16", 128, 1),
]
F16B_CONSTS = [
    ("nwfc2_1", 128, NS * 64), ("nwfc2_2", 128, NS * 64),
    ("bandc1", 96, (16 // LEVELS[1][3]) * 128),
    ("bandc2", 96, (16 // LEVELS[2][3]) * 128),
]


def _pack_offsets(spec):
    offs, col = {}, 0
    for name, rows, cols in spec:
        offs[name] = (col, rows, cols)
        col += cols
    return offs, col


F32_OFFS, F32_NCOL = _pack_offsets(F32_CONSTS)
F16A_OFFS, F16A_NCOL = _pack_offsets(F16A_CONSTS)
F16B_OFFS, F16B_NCOL = _pack_offsets(F16B_CONSTS)


def _build_program(num_devices=N_CORES):
    nc = bacc.Bacc("TRN2", target_bir_lowering=False, debug=False,
                   num_devices=num_devices)
    D = {}

    def din(name, shape, dtype=F32):
        D[name] = nc.dram_tensor(name, list(shape), dtype, kind="ExternalInput")

    din("cpak", (128, F32_NCOL))
    din("hpak0", (128, F16A_NCOL), FP16)
    din("hpak1", (128, F16B_NCOL), FP16)
    for li, (H, W, Wp, pack) in enumerate(LEVELS):
        din(f"festack{li}", (128, NB * H * Wp), FP16)
    din("anch0", (128, NB * 3))
    din("anch1", (64, NB * 3))
    din("sinargsT", (64, NB))

    out_t = nc.dram_tensor("out", [NB, NP, 78], F32, kind="ExternalOutput")

    with tile.TileContext(nc) as tc, ExitStack() as ex:
        cpool = ex.enter_context(tc.tile_pool(name="consts", bufs=1))
        state = ex.enter_context(tc.tile_pool(name="state", bufs=1))
        wk = ex.enter_context(tc.tile_pool(name="work", bufs=2))
        big = ex.enter_context(tc.tile_pool(name="big", bufs=2))
        psA = ex.enter_context(tc.tile_pool(name="psA", bufs=3, space="PSUM"))
        psB = ex.enter_context(tc.tile_pool(name="psB", bufs=2, space="PSUM"))
        psC = ex.enter_context(tc.tile_pool(name="psC", bufs=1, space="PSUM"))
        psD = ex.enter_context(tc.tile_pool(name="psD", bufs=2, space="PSUM"))

        cpak = cpool.tile([128, F32_NCOL], F32, tag="cpak", name="cpak")
        hpak0 = cpool.tile([128, F16A_NCOL], FP16, tag="hpak0", name="hpak0")
        hpak1 = cpool.tile([128, F16B_NCOL], FP16, tag="hpak1", name="hpak1")
        nc.sync.dma_start(cpak[:], D["cpak"].ap())

        def Cf(name, p0=0, pn=None, c0=0, cn=None):
            off, rows, cols = F32_OFFS[name]
            pn = rows if pn is None else pn
            cn = cols if cn is None else cn
            return cpak[p0:p0 + pn, off + c0:off + c0 + cn]

        def Ch(name, p0=0, pn=None, c0=0, cn=None):
            if name in F16A_OFFS:
                off, rows, cols = F16A_OFFS[name]
                pak = hpak0
            else:
                off, rows, cols = F16B_OFFS[name]
                pak = hpak1
            pn = rows if pn is None else pn
            cn = cols if cn is None else cn
            return pak[p0:p0 + pn, off + c0:off + c0 + cn]

        anch = []
        for ci, (p0, pn) in enumerate(PCH):
            a = state.tile([pn, NB * 3], F32, tag=f"anch{ci}", name=f"anch{ci}")
            nc.sync.dma_start(a[:], D[f"anch{ci}"].ap())
            anch.append(a)
        sarg = cpool.tile([64, NB], F32, tag="sarg", name="sarg")
        nc.sync.dma_start(sarg[:], D["sinargsT"].ap())
        nc.sync.dma_start(hpak0[:], D["hpak0"].ap())

        fst = []
        for li, (H, W, Wp, pack) in enumerate(LEVELS):
            t = cpool.tile([128, NB * H * Wp], FP16, tag=f"fst{li}",
                           name=f"fst{li}")
            nc.sync.dma_start(t[:], D[f"festack{li}"].ap())
            fst.append(t)
            if li == 0:
                nc.sync.dma_start(hpak1[:], D["hpak1"].ap())

        osts = {}
        for b in range(NB):
            for ci, (p0, pn) in enumerate(PCH):
                osts[(b, ci)] = state.tile([pn, 78], F32, tag=f"ost{b}_{ci}",
                                           name=f"ost{b}_{ci}")

        # ---------------- time MLP ----------------
        sinT = wk.tile([64, NB], F32, tag="tm_sin", name="sinT")
        nc.scalar.activation(sinT[:], sarg[:], AF.Sin)
        emb = []
        for m in range(2):
            p = psD.tile([128, NB], F32, tag="mm", name=f"p_emb{m}")
            nc.tensor.matmul(p[:], Cf("W_t1", 0, 64, m * 128, 128), sinT[:])
            x = state.tile([128, NB], F32, tag=f"emb{m}", name=f"emb{m}")
            nc.scalar.activation(x[:], p[:], AF.Identity,
                                 bias=Cf("b_t1", 0, 128, m, 1))
            sq = wk.tile([128, NB], F32, tag="tm_sq", name=f"sq{m}")
            nc.scalar.activation(sq[:], x[:], AF.Square)
            cu = wk.tile([128, NB], F32, tag="tm_cu", name=f"cu{m}")
            nc.vector.tensor_tensor(cu[:], sq[:], x[:], ALU.mult)
            nc.vector.tensor_scalar(cu[:], cu[:], 0.044715, None, ALU.mult)
            nc.vector.tensor_tensor(cu[:], cu[:], x[:], ALU.add)
            th = wk.tile([128, NB], F32, tag="tm_th", name=f"th{m}")
            nc.scalar.activation(th[:], cu[:], AF.Tanh,
                                 scale=float(np.sqrt(2.0 / np.pi)))
            nc.vector.tensor_scalar(th[:], th[:], 1.0, 0.5, ALU.add, ALU.mult)
            nc.vector.tensor_tensor(x[:], th[:], x[:], ALU.mult)
            emb.append(x)
        tmb = []
        for m in range(2):
            p = psD.tile([128, NB], F32, tag="mm", name=f"p_tmb{m}")
            for k in range(2):
                wt2 = Cf("W_t2a" if k == 0 else "W_t2b", 0, 128, m * 128, 128)
                nc.tensor.matmul(p[:], wt2, emb[k][:],
                                 start=(k == 0), stop=(k == 1))
            x = state.tile([128, NB], F32, tag=f"tmb{m}", name=f"tmb{m}")
            nc.scalar.activation(x[:], p[:], AF.Identity,
                                 bias=Cf("b_t2", 0, 128, m, 1))
            tmb.append(x)
        sil = []
        for m in range(2):
            s = wk.tile([128, NB], F32, tag=f"tm_sil{m}", name=f"sil{m}")
            nc.scalar.activation(s[:], tmb[m][:], AF.Sigmoid)
            nc.vector.tensor_tensor(s[:], s[:], tmb[m][:], ALU.mult)
            sil.append(s)
        scale1T = state.tile([64, NB], F32, tag="scale1T", name="scale1T")
        shiftT = state.tile([64, NB], F32, tag="shiftT", name="shiftT")
        for j, (dst, bias) in enumerate([(scale1T, "bstS1"), (shiftT, "bstSh")]):
            p = psD.tile([64, NB], F32, tag="mm", name=f"p_ss{j}")
            for k in range(2):
                wst = Cf("W_sta" if k == 0 else "W_stb", 0, 128, j * 64, 64)
                nc.tensor.matmul(p[:], wst, sil[k][:],
                                 start=(k == 0), stop=(k == 1))
            nc.scalar.activation(dst[:], p[:], AF.Identity, bias=Cf(bias))
        tokT = state.tile([64, NB], F32, tag="tokT", name="tokT")
        ptk = psD.tile([64, NB], F32, tag="mm", name="p_tok")
        for k in range(2):
            wtc = Cf("W_tca" if k == 0 else "W_tcb")
            nc.tensor.matmul(ptk[:], wtc, tmb[k][:], start=(k == 0), stop=(k == 1))
        nc.scalar.activation(tokT[:], ptk[:], AF.Identity, bias=Cf("b_tc"))

        # ---------------- per-level helpers ----------------
        def gen_ab(li, W, scaleW, c0=0, nb=NB):
            """Batched trig across items [c0, c0+nb): per-chunk (aC,bC,base,g)."""
            res = []
            for ci, (p0, pn) in enumerate(PCH):
                A = anch[ci]
                lo, hi = c0 * 3, (c0 + nb) * 3
                sn = wk.tile([pn, nb], F32, tag=f"sn{ci}", name=f"sn{ci}_{li}_{c0}")
                cs = wk.tile([pn, nb], F32, tag=f"cs{ci}", name=f"cs{ci}_{li}_{c0}")
                nc.scalar.activation(sn[:], A[:, lo + 2:hi:3], AF.Sin,
                                     scale=math.pi)
                nc.scalar.activation(cs[:], A[:, lo + 2:hi:3], AF.Sin,
                                     scale=-math.pi, bias=Cf("halfpi", 0, pn))
                g = wk.tile([pn, nb], F32, tag=f"g{ci}", name=f"g{ci}_{li}_{c0}")
                nc.vector.reciprocal(g[:], sn[:])
                nc.vector.tensor_tensor(g[:], cs[:], g[:], ALU.mult)
                nc.vector.tensor_scalar(g[:], g[:], 1000.0, -1000.0,
                                        ALU.min, ALU.max)
                nc.vector.tensor_scalar(g[:], g[:], ALPHA, None, ALU.mult)
                base = wk.tile([pn, nb], F32, tag=f"bs{ci}", name=f"bs{ci}_{li}_{c0}")
                nc.vector.tensor_tensor(base[:], A[:, lo:hi:3], g[:], ALU.mult)
                nc.vector.tensor_tensor(base[:], A[:, lo + 1:hi:3], base[:],
                                        ALU.subtract)
                aC = wk.tile([pn, nb], F32, tag=f"aC{ci}", name=f"aC{ci}_{li}_{c0}")
                bC = wk.tile([pn, nb], F32, tag=f"bC{ci}", name=f"bC{ci}_{li}_{c0}")
                nc.vector.tensor_scalar(aC[:], base[:], scaleW, None, ALU.mult)
                nc.vector.tensor_scalar(bC[:], g[:], scaleW, None, ALU.mult)
                res.append((aC, bC, base, g))
            return res

        def head_mm(li, wname, bias, src, relu=True, out_p=64, tag="hd"):
            # f32 chain: moving src f32 + f32 stationary weights
            p = psD.tile([128, 2 * NP], F32, tag="mm", name=f"p{tag}_{li}")
            nc.tensor.matmul(p[0:out_p, :], Cf(wname), src[:])
            o = wk.tile([out_p, 2 * NP], F32, tag=f"hd_{tag}",
                        name=f"{tag}o_{li}")
            if HEADS_DVE:
                if relu:
                    nc.vector.tensor_scalar(o[:], p[0:out_p, :],
                                            Cf(bias, 0, out_p), 0.0,
                                            ALU.add, ALU.max)
                else:
                    nc.vector.tensor_scalar(o[:], p[0:out_p, :],
                                            Cf(bias, 0, out_p), None, ALU.add)
            else:
                nc.scalar.activation(o[:], p[0:out_p, :],
                                     AF.Relu if relu else AF.Identity,
                                     bias=Cf(bias, 0, out_p))
            return o

        def emit_heads(li, W, fHp, is_last):
            for bp in range(NB // 2):
                fH = fHp[bp]
                r1 = head_mm(li, "Wf_r1", "b_r1", fH, tag=f"r1{bp}")
                r2 = head_mm(li, "Wf_r2", "b_r2", r1, tag=f"r2{bp}")
                regT = head_mm(li, "Wf_reg", "b_reg", r2, relu=False, out_p=76,
                               tag=f"rg{bp}")
                clsT = None
                if is_last:
                    c1 = head_mm(li, "Wf_c1", "b_c1", fH, tag=f"c1{bp}")
                    c2 = head_mm(li, "Wf_c2", "b_c2", c1, tag=f"c2{bp}")
                    clsT = head_mm(li, "Wf_cls", "b_cls", c2, relu=False,
                                   out_p=2, tag=f"cl{bp}")
                for ci, (p0, pn) in enumerate(PCH):
                    pt2 = psD.tile([128, 2 * 76], F32, tag="mm",
                                   name=f"p_rt{bp}{ci}_{li}")
                    for u in range(2):
                        nc.tensor.transpose(
                            pt2[0:pn, u * 76:u * 76 + 76],
                            regT[:, u * NP + p0:u * NP + p0 + pn],
                            Cf("ident", 0, 76, 0, 76))
                    rn2 = state.tile([pn, 2 * 76], F32, tag=f"regn{bp}_{ci}",
                                     name=f"regn{bp}{ci}_{li}")
                    nc.vector.tensor_copy(rn2[:], pt2[0:pn, :])
                    A = anch[ci]
                    for u in range(2):
                        b = bp * 2 + u
                        nc.vector.tensor_tensor(A[:, b * 3:(b + 1) * 3],
                                                A[:, b * 3:(b + 1) * 3],
                                                rn2[:, u * 76:u * 76 + 3],
                                                ALU.add)
                        if is_last:
                            ost = osts[(b, ci)]
                            nc.vector.tensor_copy(ost[:, 2:5],
                                                  A[:, b * 3:(b + 1) * 3])
                            nc.vector.tensor_copy(ost[:, 5:6],
                                                  rn2[:, u * 76 + 3:u * 76 + 4])
                            _CACHE.setdefault("regn", {})[(b, ci)] = (rn2, u)
                    if is_last:
                        ptc = psD.tile([128, 4], F32, tag="mm",
                                       name=f"p_ct{bp}{ci}")
                        for u in range(2):
                            nc.tensor.transpose(
                                ptc[0:pn, u * 2:u * 2 + 2],
                                clsT[:, u * NP + p0:u * NP + p0 + pn],
                                Cf("ident", 0, 2, 0, 2))
                        for u in range(2):
                            b = bp * 2 + u
                            nc.vector.tensor_copy(osts[(b, ci)][:, 0:2],
                                                  ptc[0:pn, u * 2:u * 2 + 2])
                if is_last:
                    abf = gen_ab(li, W, 1.0, c0=bp * 2, nb=2)
                    for u in range(2):
                        b = bp * 2 + u
                        for ci, (p0, pn) in enumerate(PCH):
                            _, _, base, g = abf[ci]
                            rn2, uu = _CACHE["regn"][(b, ci)]
                            ost = osts[(b, ci)]
                            nc.vector.tensor_scalar(ost[:, 6:78],
                                                    Cf("qfrep", 0, pn),
                                                    g[:, u:u + 1],
                                                    base[:, u:u + 1],
                                                    ALU.mult, ALU.add)
                            nc.vector.tensor_tensor(
                                ost[:, 6:78], ost[:, 6:78],
                                rn2[:, uu * 76 + 4:uu * 76 + 76], ALU.add)
                            nc.sync.dma_start(out_t.ap()[b, p0:p0 + pn, :],
                                              ost[:])

        # ---------------- main loop ----------------
        pending = [None]
        for li, (H, W, Wp, pack) in enumerate(LEVELS):
            G = NS // pack
            nvar = 16 // pack
            is_last = li == len(LEVELS) - 1
            fstL = fst[li]
            ntile = (G + GPT - 1) // GPT
            y0t, _, _, _ = _level_ytab(H)
            PW = pack * Wp
            npairs = (G + 1) // 2

            # --- RC for ALL items first: independent of anchors, fills the
            # level-boundary pipeline bubble ---
            rcsbs_all = {}
            rci = 0
            for b in range(NB):
                for t in range(ntile):
                    g_lo = t * GPT
                    g_hi = min(G, g_lo + GPT)
                    rcp = psB.tile([128, 512], F32, tag="rc",
                                   name=f"rc{b}_{li}_{t}")
                    for j in range(pack):
                        g = g_lo
                        while g < g_hi:
                            s = j * G + g
                            y = int(y0t[s])
                            glen = 1
                            while (g + glen < g_hi
                                   and int(y0t[s + glen]) == y):
                                glen += 1
                            nc.tensor.matmul(
                                rcp[j * Wp:(j + 1) * Wp,
                                    (g - g_lo) * 64:(g - g_lo + glen) * 64],
                                fstL[:, (b * H + y) * Wp:
                                     (b * H + y + 1) * Wp],
                                Ch(f"nwfc2_{li}", 0, 128, s * 64, glen * 64),
                                start=True, stop=True,
                                tile_position=(0, j * Wp))
                            g += glen
                    ng = g_hi - g_lo
                    rcsb = big.tile([128, 512],
                    FP16 if GATHER == "dual" else F32,
                    tag="rcsb", bufs=22,
                                    name=f"rcsb{b}_{li}_{t}")
                    use_act = RC_ACT[li]
                    if use_act is None:
                        use_act = rci % 2 == 0
                    if use_act:
                        nc.scalar.activation(rcsb[0:PW, 0:ng * 64],
                                             rcp[0:PW, 0:ng * 64], AF.Copy)
                    else:
                        nc.vector.tensor_copy(rcsb[0:PW, 0:ng * 64],
                                              rcp[0:PW, 0:ng * 64])
                    rcsl = None
                    if GATHER == "dual":
                        rcsl = big.tile([128, 512], FP16, tag="rcsl", bufs=22,
                                        name=f"rcsl{b}_{li}_{t}")
                        nc.vector.tensor_tensor(rcsl[0:PW, 0:ng * 64],
                                                rcp[0:PW, 0:ng * 64],
                                                rcsb[0:PW, 0:ng * 64],
                                                ALU.subtract)
                    rci += 1
                    rcsbs_all.setdefault(b, []).append((rcsb, rcsl))

            # previous level's heads (anchor updates) overlap this RC block
            if pending[0] is not None:
                pending[0]()
                pending[0] = None

            ab = gen_ab(li, W, float(W - 1))

            # --- phases A+B per item-pair (single shared fps bank:
            # bp0 at partitions 0:64, bp1 at 64:128) ---
            fpsT = psC.tile([128, 2 * NP], F32, tag="fps", name=f"fpsT_{li}")
            fT16s = []
            est2 = {}
            vn2 = {}
            for bp in range(NB // 2):
                r0 = 64 * (bp % 2)
                for u in range(2):
                    b = bp * 2 + u
                    rcsbs = rcsbs_all[b]
                    # A: xf -> xi/frac interleaved -> transpose -> xfif fp16
                    trp = psD.tile([96, NP], F32, tag="mm", name=f"trp{b}_{li}")
                    for ci, (p0, pn) in enumerate(PCH):
                        aC, bC, _, _ = ab[ci]
                        pre = wk.tile([pn, 96], F32, tag=f"pre{ci}", bufs=2,
                                      name=f"pre{b}{ci}_{li}")
                        xf = wk.tile([pn, NS], F32, tag=f"xf{ci}", bufs=2,
                                     name=f"xf{b}{ci}_{li}")
                        nc.vector.tensor_scalar(xf[:], Cf(f"qrep{li}", 0, pn),
                                                bC[:, b:b + 1], aC[:, b:b + 1],
                                                ALU.mult, ALU.add)
                        nc.vector.tensor_scalar(xf[:], xf[:], float(W + 1),
                                                -2.0, ALU.min, ALU.max)
                        # xi = round(xf) via f32 2^23 add/sub; two separate
                        # instructions so the intermediate is rounded to f32
                        # in SBUF (a fused two-op chain may keep extra
                        # precision on HW and break the exact-integer split)
                        rtmp = wk.tile([pn, NS], F32, tag=f"rt{ci}", bufs=2,
                                       name=f"rt{b}{ci}_{li}")
                        nc.vector.tensor_scalar(rtmp[:], xf[:],
                                                8388608.0, None, ALU.add)
                        nc.vector.tensor_scalar(pre[:, 0:72:2], rtmp[:],
                                                8388608.0, None, ALU.subtract)
                        nc.vector.tensor_tensor(pre[:, 1:72:2], xf[:],
                                                pre[:, 0:72:2], ALU.subtract)
                        nc.vector.memset(pre[:, 72:96], 0.0)
                        nc.tensor.transpose(trp[:, p0:p0 + pn], pre[:],
                                            Cf("ident", 0, pn, 0, pn))
                    xfif = big.tile([96, NP], FP16, tag="xfif", bufs=3,
                                    name=f"xfif{b}_{li}")
                    nc.vector.tensor_copy(xfif[:], trp[:])

                    # tents (pair pipeline, stage2 lagged by one pair)
                    def s2(g, pvt, uu, last, r0=r0, u=u, rcsbs=rcsbs):
                        hi, lo = rcsbs[g // GPT]
                        c0 = (g % GPT) * 64
                        nc.tensor.matmul(
                            fpsT[r0:r0 + 64, u * NP:(u + 1) * NP],
                            hi[0:PW, c0:c0 + 64],
                            pvt[0:PW, uu * NP:(uu + 1) * NP],
                            start=(g == 0), stop=(last and lo is None))
                        if lo is not None:
                            nc.tensor.matmul(
                                fpsT[r0:r0 + 64, u * NP:(u + 1) * NP],
                                lo[0:PW, c0:c0 + 64],
                                pvt[0:PW, uu * NP:(uu + 1) * NP],
                                start=False, stop=last)

                    pend = None
                    pi = 0
                    for g0 in range(0, G, 2):
                        npair = min(2, G - g0)
                        xfps = psA.tile([128, 2 * NP], F32, tag="xfps",
                                        name=f"xfps{b}_{li}_{g0}")
                        for uu in range(npair):
                            g = g0 + uu
                            k = (g * pack) // 16
                            v = g - k * nvar
                            nc.tensor.matmul(
                                xfps[0:PW, uu * NP:(uu + 1) * NP],
                                Ch(f"bandc{li}", 32 * k, 32, v * 128, PW),
                                xfif[32 * k:32 * k + 32, :],
                                start=True, stop=True,
                                tile_position=(32 * k, 0))
                        d1 = big.tile([128, 2 * NP], F32, tag="d1", bufs=3,
                                      name=f"d1{b}_{li}_{g0}")
                        mod = ABS_DVE_MOD[li]
                        if mod and pi % mod == 0:
                            nc.vector.tensor_scalar(
                                d1[0:PW, 0:npair * NP],
                                xfps[0:PW, 0:npair * NP],
                                Cf(f"negiota{li}", 0, PW), 0.0,
                                ALU.add, ALU.abs_max)
                        else:
                            nc.scalar.activation(d1[0:PW, 0:npair * NP],
                                                 xfps[0:PW, 0:npair * NP],
                                                 AF.Abs,
                                                 bias=Cf(f"negiota{li}", 0, PW))
                        vt = big.tile([128, 2 * NP],
                                      FP16 if GATHER == "dual" else F32,
                                      tag="vt", bufs=4,
                                      name=f"vt{b}_{li}_{g0}")
                        mm_ = MIN_POOL_MOD[li]
                        if mm_ > 0 and pi % mm_ == 0:
                            nc.gpsimd.tensor_scalar(vt[0:PW, 0:npair * NP],
                                                    d1[0:PW, 0:npair * NP],
                                                    1.0, 0.0,
                                                    ALU.subtract, ALU.min)
                        else:
                            nc.vector.tensor_scalar(vt[0:PW, 0:npair * NP],
                                                    d1[0:PW, 0:npair * NP],
                                                    1.0, 0.0,
                                                    ALU.subtract, ALU.min)
                        if pend is not None:
                            pg0, pn_, pvt = pend
                            for uu in range(pn_):
                                s2(pg0 + uu, pvt, uu, False)
                        pend = (g0, npair, vt)
                        pi += 1
                    pg0, pn_, pvt = pend
                    for uu in range(pn_):
                        g = pg0 + uu
                        s2(g, pvt, uu, g == G - 1)

                fTf = big.tile([64, 2 * NP], F32, tag="fTf", bufs=2,
                               name=f"fTf_{bp}_{li}")
                nc.scalar.activation(fTf[:], fpsT[r0:r0 + 64, :], AF.Relu,
                                     bias=Cf("b_fc"))
                for u in range(2):
                    b = bp * 2 + u
                    nc.vector.tensor_scalar(fTf[:, u * NP:(u + 1) * NP],
                                            fTf[:, u * NP:(u + 1) * NP],
                                            tokT[:, b:b + 1], None, ALU.add)
                fT16 = big.tile([64, 2 * NP], FP16, tag="fT16", bufs=2,
                                name=f"fT16_{bp}_{li}")
                nc.vector.tensor_copy(fT16[:], fTf[:])
                fT16s.append((fT16, fTf))

            # --- phase C1: q/k/v + scores + exp (item-paired) ---
            for bp in range(NB // 2):
                fT16, fTf = fT16s[bp]
                qp = psD.tile([64, 2 * NP], F32, tag="mm", name=f"qp{bp}_{li}")
                nc.tensor.matmul(qp[:], Cf("Wf_q"), fTf[:])
                qT = wk.tile([64, 2 * NP], F32, tag="qT", name=f"qT{bp}_{li}")
                nc.vector.tensor_scalar(qT[:], qp[:], 0.125, None, ALU.mult)
                kp = psD.tile([64, 2 * NP], F32, tag="mm", name=f"kp{bp}_{li}")
                nc.tensor.matmul(kp[:], Cf("Wf_k"), fTf[:])
                kT = wk.tile([64, 2 * NP], F32, tag="kT", name=f"kT{bp}_{li}")
                nc.vector.tensor_copy(kT[:], kp[:])
                for ci, (p0, pn) in enumerate(PCH):
                    vp2 = psD.tile([128, 128], F32, tag="mm",
                                   name=f"vp{bp}{ci}_{li}")
                    sp2 = psD.tile([128, 2 * NP], F32, tag="mm",
                                   name=f"sp{bp}{ci}_{li}")
                    for u in range(2):
                        nc.tensor.matmul(vp2[0:pn, u * 64:(u + 1) * 64],
                                         fTf[:, u * NP + p0:u * NP + p0 + pn],
                                         Cf("Wf_v"))
                        nc.tensor.matmul(sp2[0:pn, u * NP:(u + 1) * NP],
                                         kT[:, u * NP + p0:u * NP + p0 + pn],
                                         qT[:, u * NP:(u + 1) * NP])
                    vtl = wk.tile([pn, 128], FP16, tag=f"vn{ci}", bufs=2,
                                  name=f"vn{bp}{ci}_{li}")
                    nc.vector.tensor_copy(vtl[:], vp2[0:pn, :])
                    vn2[(bp, ci)] = vtl
                    e = wk.tile([pn, 2 * NP], FP16, tag=f"est{ci}", bufs=2,
                                name=f"est{bp}{ci}_{li}")
                    nc.scalar.activation(e[:], sp2[0:pn, :], AF.Exp)
                    est2[(bp, ci)] = e

            # --- phase C2: softmax denom + attn out + FiLM (item-paired) ---
            fHp = []
            for bp in range(NB // 2):
                rbc2 = wk.tile([64, 2 * NP], F32, tag="rbc", bufs=2,
                               name=f"rbc{bp}_{li}")
                for u in range(2):
                    b = bp * 2 + u
                    zp = psD.tile([1, NP], F32, tag="mm", name=f"zp{b}_{li}")
                    for ci, (p0, pn) in enumerate(PCH):
                        nc.tensor.matmul(zp[:], Ch("ones16", 0, pn),
                                         est2[(bp, ci)][:, u * NP:(u + 1) * NP],
                                         start=(ci == 0), stop=(ci == 1))
                    rrow = wk.tile([1, NP], F32, tag="rrow",
                                   name=f"rrow{b}_{li}")
                    nc.vector.reciprocal(rrow[:], zp[:])
                    nc.gpsimd.partition_broadcast(rbc2[:, u * NP:(u + 1) * NP],
                                                  rrow[0:1, :], channels=64)
                avp2 = psD.tile([64, 2 * NP], F32, tag="mm", name=f"av{bp}_{li}")
                for u in range(2):
                    for ci in range(2):
                        nc.tensor.matmul(
                            avp2[:, u * NP:(u + 1) * NP],
                            vn2[(bp, ci)][:, u * 64:(u + 1) * 64],
                            est2[(bp, ci)][:, u * NP:(u + 1) * NP],
                            start=(ci == 0), stop=(ci == 1))
                avsb = wk.tile([64, 2 * NP], FP16, tag="avsb",
                               name=f"av{bp}_{li}")
                nc.vector.tensor_copy(avsb[:], avp2[:])
                opp = psD.tile([64, 2 * NP], F32, tag="mm", name=f"opp{bp}_{li}")
                nc.tensor.matmul(opp[:], Ch("W_o"), avsb[:])
                t12 = wk.tile([64, 2 * NP], F32, tag="attnt",
                              name=f"t1{bp}_{li}")
                nc.vector.tensor_tensor(t12[:], opp[:], rbc2[:], ALU.mult)
                nc.vector.tensor_tensor(t12[:], fT16s[bp][1][:], t12[:],
                                        ALU.add)
                fH = big.tile([64, 2 * NP], F32, tag="fH", bufs=2,
                              name=f"fH{bp}_{li}")
                for u in range(2):
                    b = bp * 2 + u
                    nc.vector.tensor_scalar(fH[:, u * NP:(u + 1) * NP],
                                            t12[:, u * NP:(u + 1) * NP],
                                            scale1T[:, b:b + 1],
                                            shiftT[:, b:b + 1],
                                            ALU.mult, ALU.add)
                fHp.append(fH)

            pending[0] = (lambda li=li, W=W, fHp=fHp, is_last=is_last:
                          emit_heads(li, W, fHp, is_last))

        pending[0]()

    nc.compile()
    _CACHE.pop("regn", None)
    return nc


def _host_inputs(inp_slice, nwfc_l):
    m = {}
    feats = [inp_slice["feat2"], inp_slice["feat1"], inp_slice["feat0"]]
    for li, (H, W, Wp, pack) in enumerate(LEVELS):
        f = np.asarray(feats[li], np.float32)  # (NB, 64, H, W)
        y1 = np.minimum(np.arange(H) + 1, H - 1)
        top = np.zeros((64, NB, H, Wp), np.float32)
        bot = np.zeros((64, NB, H, Wp), np.float32)
        top[:, :, :, :W] = f.transpose(1, 0, 2, 3)
        bot[:, :, :, :W] = f[:, :, y1, :].transpose(1, 0, 2, 3)
        st = np.concatenate([top.reshape(64, -1), bot.reshape(64, -1)], axis=0)
        m[f"festack{li}"] = st.astype(np.float16)

    w = {k: np.asarray(v, np.float32) for k, v in inp_slice.items()
         if k.startswith(("W_", "b_"))}

    cp = np.zeros((128, F32_NCOL), np.float32)

    def put32(name, val):
        off, r, c = F32_OFFS[name]
        val = np.asarray(val, np.float32)
        if val.ndim == 1:
            val = val.reshape(-1, 1)
        cp[0:val.shape[0], off:off + val.shape[1]] = val

    for li in range(3):
        qs = Q_S[_s_of_t(li)]
        put32(f"qrep{li}", np.broadcast_to(qs[None, :], (128, NS)))
        put32(f"negiota{li}",
              -(np.arange(128, dtype=np.float32) % LEVELS[li][2]))
    put32("qfrep", np.broadcast_to(QF_R[None, :], (128, NR)))
    put32("halfpi", np.full((128, 1), math.pi / 2.0))
    put32("ident", np.eye(128))
    put32("W_t1", w["W_t1"])
    put32("b_t1", np.ascontiguousarray(w["b_t1"].reshape(2, 128).T))
    put32("W_t2a", w["W_t2"][:128]); put32("W_t2b", w["W_t2"][128:])
    put32("b_t2", np.ascontiguousarray(w["b_t2"].reshape(2, 128).T))
    put32("W_sta", w["W_st"][:128]); put32("W_stb", w["W_st"][128:])
    put32("bstS1", w["b_st"][:64] + 1.0)
    put32("bstSh", w["b_st"][64:])
    put32("W_tca", w["W_tc"][:128]); put32("W_tcb", w["W_tc"][128:])
    for nm in ["b_tc", "b_fc", "b_c1", "b_c2", "b_r1", "b_r2", "b_cls", "b_reg"]:
        put32(nm, w[nm])
    for nm in ["c1", "c2", "r1", "r2", "cls", "reg", "q", "k", "v"]:
        put32(f"Wf_{nm}", w[f"W_{nm}"])
    m["cpak"] = cp

    hpa = np.zeros((128, F16A_NCOL), np.float16)
    hpb = np.zeros((128, F16B_NCOL), np.float16)

    def put16(name, val):
        offs, hp = (F16A_OFFS, hpa) if name in F16A_OFFS else (F16B_OFFS, hpb)
        off, r, c = offs[name]
        val = np.asarray(val)
        if val.ndim == 1:
            val = val.reshape(-1, 1)
        hp[0:val.shape[0], off:off + val.shape[1]] = val.astype(np.float16)

    for li in range(3):
        put16(f"nwfc2_{li}", nwfc_l[li])
        put16(f"bandc{li}", _bandc(LEVELS[li][2], LEVELS[li][3]))
    for nm in ["W_q", "W_k", "W_v", "W_o", "W_c1", "W_c2", "W_r1", "W_r2",
               "W_cls", "W_reg"]:
        put16(nm, w[nm])
    put16("ones16", np.ones((128, 1)))
    m["hpak0"] = hpa
    m["hpak1"] = hpb

    a = np.asarray(inp_slice["inputs"], np.float32)  # (NB, NP, 3)
    a0 = np.zeros((128, NB * 3), np.float32)
    a1 = np.zeros((64, NB * 3), np.float32)
    for b in range(NB):
        a0[:, b * 3:(b + 1) * 3] = a[b, 0:128, :]
        a1[:, b * 3:(b + 1) * 3] = a[b, 128:192, :]
    m["anch0"] = a0
    m["anch1"] = a1

    half = FC // 2
    freqs = np.exp(np.arange(half, dtype=np.float32)
                   * (-math.log(10000.0) / (half - 1)))
    ang = np.asarray(inp_slice["t"]).astype(np.float32)[:, None] * freqs[None, :]
    full = np.concatenate([ang, ang + math.pi / 2.0], axis=1)
    full = np.mod(full + math.pi, 2.0 * math.pi) - math.pi
    m["sinargsT"] = np.ascontiguousarray(full.T).astype(np.float32)
    return {k: np.ascontiguousarray(np.asarray(v)) for k, v in m.items()}


def make_in_maps(inputs):
    inputs = {k: np.asarray(v) for k, v in inputs.items()}
    W_fc = np.asarray(inputs["W_fc"], np.float32)
    nwfc_l = [_nwfc2(W_fc, H) for H, W, Wp, pack in LEVELS]
    in_maps = []
    for c in range(N_CORES):
        sl = slice(c * NB, (c + 1) * NB)
        inp_slice = {k: (v[sl] if k in ("feat0", "feat1", "feat2", "inputs", "t")
                         else v) for k, v in inputs.items()}
        in_maps.append(_host_inputs(inp_slice, nwfc_l))
    return in_maps


def kernel(**inputs):
    if "prog" not in _CACHE:
        _CACHE["prog"] = _build_program()
    nc = _CACHE["prog"]
    in_maps = make_in_maps(inputs)
    res = bass_utils.run_bass_kernel_spmd(nc, in_maps,
                                          core_ids=list(range(N_CORES)))
    out = np.concatenate([res.results[c]["out"] for c in range(N_CORES)], axis=0)
    return np.ascontiguousarray(out.astype(np.float32))
